# revision 70
# baseline (speedup 1.0000x reference)
"""Trainium2 Bass kernel for a ViT-style transformer block (B=4, N=1370, C=1024).

Sharding: 8 cores = 4 batches x 2 token-halves. Each core runs the full block
for its 685 query tokens; K/V are computed for all 1370 tokens of its batch
(no collectives). The token-half selection is done by rolling the token axis
on the host so every core runs an identical program on tokens [0, 685).

Key optimizations over the feature-major fp8 baseline (~320us -> ~223us):
  - All projection GEMMs (QKV, attn-out, fc1, fc2) in fp8e4m3 DoubleRow.
    This e4m3 flavor saturates at 240 (with inf), so activation scales are
    chosen conservatively (Q/K x32, V x32, O x32).
  - Attention scores ALSO run fp8 DoubleRow: QKV weight columns are permuted
    on the host so Q^T/K^T land in a [32, 2, tokens] pair layout per head
    (channel d of head h at partition 32*(h%4) + d%32, pair j = d//32),
    halving score matmul cost for free. Keys are zero-padded to 11*128 so no
    ragged tiles exist anywhere in the attention core.
  - A@V is re-oriented to out[queries, DH+1] with the softmax probabilities
    as the stationary operand: output free size is 65 instead of 685 per
    instruction, and the softmax denominator (an alpha-column in the fp8 V
    operand) lands on the same partitions as its queries, so normalization
    is a per-partition reciprocal + one stride-0-broadcast multiply -- no
    DMA round trip. Normalized O transposes back to feature-major via cheap
    PE transposes; the transposes for query tiles 0-3 are injected into the
    second attention half (its narrower score PSUM tiles free 2 banks),
    filling otherwise-idle PE time there.
  - softmax exp splits across TWO engines: ACT computes exact Exp for ~6 of
    11 key tiles (groups 0, 2); DVE computes a Schraudolph bit-trick exp
    (int16 bits = A*s + B reinterpreted as bf16, ~4% rel err) for the rest.
    Attention-output error is invisible under the 1e-5 layer scale.
  - LayerNorm gains/biases fold into the adjacent projection weights/biases
    on the host; x is loaded in bf16 and stats come from ones-matmuls on the
    PE; the normalize is 2 passes (GPSIMD subtract with stride-0 broadcast
    mean, DVE multiply by broadcast rstd) writing fp8 directly.
  - The attention residual x1 is kept in bf16 (~4e-4 relative output error,
    well under the 2e-2 gate) which shortens the LN2 chain.
  - Engine balance: Q/K/V PSUM evacuations on ACT (Identity with bias AP),
    wave-2 evacs on DVE so the ACT queue is clear when exp starts; fc1+gelu
    run m-quadded (bias-free fast path) overlapping LN2's tail; fc2 weights
    prefetch during phase D on an otherwise idle DMA window. A DMA holds its
    issuing sequencer for the whole transfer, so the x-input and w_v loads
    issue from the (then-idle) ACT queue while SP streams the other weights
    in parallel.
The emission order keeps the PE queue full (the cost model's PE clock drops
after idle): scores of item i+1 interleave with A@V chains of item i via a
pending-work pump. A post-scheduling pass legalizes multi-wait instructions
for this walrus build (one sync wait per instruction).
"""

import numpy as np
import ml_dtypes

import concourse.bass as bass
import concourse.mybir as mybir
import concourse.tile as tile
from concourse.bass_utils import run_bass_kernel_spmd

B, N, C = 4, 1370, 1024
H, DH, HID = 16, 64, 4096
P = 128
CT = C // P            # 8 feature tiles
HT = HID // P          # 32 hidden tiles
NCORES = 8
Q = N // 2             # 685 query tokens per core
KT = (N + P - 1) // P  # 11 key-token tiles (last has 90 rows)
EPS = 1e-5

F32 = mybir.dt.float32
F32R = mybir.dt.float32r
BF16 = mybir.dt.bfloat16
F8 = mybir.dt.float8e4
I16 = mybir.dt.int16
F8NP = mybir.dt.np(F8)

NP = 1408            # N padded to a full 11*128 keys (pad keys are zero)
QP = 688             # Q padded to 16 (fp8 DoubleRow pair-stride rule)

WS_QK = 32.0         # fp8 scale for Q/K projections (e4m3 max is 240!)
WS_V = 32.0          # fp8 scale for V / value path
ALPHA = 1.0          # vaug ones-column value; O comes out at WS_V/ALPHA
TS = WS_V / ALPHA    # scale of the normalized attention output (64)
WS_PR = 128.0        # fp8 scale for w_proj
WS_F = 256.0         # fp8 scale for fc1/fc2
SC_EXP = (DH ** -0.5) / (WS_QK * WS_QK)
EXP_A = 128.0 / np.log(2.0)   # Schraudolph bf16 exp: bits = A*x + B
EXP_B = 16256.0 - 4.0

DEBUG_DUMPS = False
ADD = mybir.AluOpType.add
SUB = mybir.AluOpType.subtract
MUL = mybir.AluOpType.mult
AF = mybir.ActivationFunctionType
DR = mybir.MatmulPerfMode.DoubleRow

QCH = [(0, 512), (512, Q - 512)]        # query chunks (attention, proj)
QCM = [(0, 343), (343, 342)]            # LN2 chunks (disjoint)
QCF = [(0, 343), (342, 343)]            # fc1/fc2 chunks (overlap col 342 so
                                        # gelu needs no ragged-pad memset)
LN1_DMA = [(0, 512), (512, 512), (1024, N - 1024)]
QT_ALL = [(i * P, min(P, Q - i * P)) for i in range((Q + P - 1) // P)]  # 6
GROUPS = [[0, 1, 2], [3, 4], [5, 6, 7], [8, 9, 10]]


def _fbc(ap, reps):
    """Broadcast an AP [P, n] -> [P, reps, n] via a stride-0 middle dim."""
    a = [list(x) for x in ap.ap]
    return bass.AP(tensor=ap.tensor, offset=ap.offset,
                   ap=[a[0], [0, reps]] + a[1:])


_WAIT_EXEMPT = {
    "InstEventSemaphore", "InstNoOp",
    "InstCall", "InstBranchHint", "InstHalt", "InstCollectiveCompute",
}


def _legalize_matmul_waits(nc):
    """This walrus build allows only ONE sync wait per compute instruction.
    Move extra waits onto NoOps inserted immediately before the instruction
    (same engine stream position => identical ordering semantics)."""
    nid = [0]
    for fn in nc.m.functions:
        for blk in fn.blocks:
            insts = blk.instructions
            i = 0
            while i < len(insts):
                ins = insts[i]
                tname = type(ins).__name__
                si = getattr(ins, "sync_info", None)
                if (tname not in _WAIT_EXEMPT and tname.startswith("Inst")
                        and si is not None and len(si.on_wait) > 1):
                    waits = list(si.on_wait)
                    for w in waits[:-1]:
                        nop = mybir.InstNoOp(
                            name=f"I-mmwait-{nid[0]}", engine=ins.engine,
                            ins=[], outs=[],
                            sync_info=mybir.SyncInfo(on_wait=[w],
                                                     on_update=[]))
                        nid[0] += 1
                        insts.insert(i, nop)
                        i += 1
                    ins.sync_info = mybir.SyncInfo(on_wait=[waits[-1]],
                                                   on_update=si.on_update)
                i += 1


def _build_program(fc1_bias_free=True):
    nc = bass.Bass()
    d = {}
    d["xt"] = nc.declare_dram_parameter("xt", [3, P, CT, 512], BF16,
                                        isOutput=False)
    d["xqb"] = nc.declare_dram_parameter("xqb", [P, CT, Q], BF16, isOutput=False)
    d["ident"] = nc.declare_dram_parameter("ident", [P, P], BF16, isOutput=False)
    d["wqk"] = nc.declare_dram_parameter("wqk", [P, 16, CT, P], F8, isOutput=False)
    d["bqk"] = nc.declare_dram_parameter("bqk", [P, 16], F32, isOutput=False)
    d["wv"] = nc.declare_dram_parameter("wv", [P, CT, C], F8, isOutput=False)
    d["wproj"] = nc.declare_dram_parameter("wproj", [P, CT, C], F8, isOutput=False)
    d["g1s"] = nc.declare_dram_parameter("g1s", [P, CT], F32, isOutput=False)
    d["wfc1"] = nc.declare_dram_parameter("wfc1", [P, CT, HID], F8, isOutput=False)
    d["bfc1"] = nc.declare_dram_parameter("bfc1", [P, HT], F32, isOutput=False)
    d["wfc2"] = nc.declare_dram_parameter("wfc2", [CT, P, HT, P], F8, isOutput=False)
    d["g2s"] = nc.declare_dram_parameter("g2s", [P, CT], F32, isOutput=False)
    d["bfc2g"] = nc.declare_dram_parameter("bfc2g", [P, CT], F32, isOutput=False)
    out_d = nc.declare_dram_parameter("out", [P, CT, Q], F32, isOutput=True)
    dbg = {}
    if DEBUG_DUMPS:
        dbg["z1"] = nc.declare_dram_parameter("dbg_z1", [P, CT, NP], F8, isOutput=True)
        dbg["QT"] = nc.declare_dram_parameter("dbg_QT", [P, 4, 2, QP], F8, isOutput=True)
        dbg["KT"] = nc.declare_dram_parameter("dbg_KT", [P, 4, 2, NP], F8, isOutput=True)
        dbg["va"] = nc.declare_dram_parameter("dbg_va", [P, KT, H, DH + 1], F8, isOutput=True)
        dbg["Ob"] = nc.declare_dram_parameter("dbg_Ob", [P, 6, H, DH], BF16, isOutput=True)
        dbg["oTT"] = nc.declare_dram_parameter("dbg_oTT", [P, CT, QP], F8, isOutput=True)
        dbg["x1T"] = nc.declare_dram_parameter("dbg_x1T", [P, CT, Q], F32, isOutput=True)
        dbg["h2T"] = nc.declare_dram_parameter("dbg_h2T", [P, CT, QP], F8, isOutput=True)

    with tile.TileContext(nc) as tc:
        with tc.tile_pool(name="const", bufs=1) as const:
            onesb = const.tile([P, P], BF16)
            nc.vector.memset(onesb, 1.0)
            eps_sb = const.tile([P, 1], F32)
            nc.vector.memset(eps_sb, EPS)
            ident = const.tile([P, P], BF16)
            _deferred_dmas = [(ident, d["ident"])]

            def load_const(name, shape):
                t = const.tile(shape, F32, tag=f"const_{name}")
                _deferred_dmas.append((t, d[name]))
                return t

            bqk_sb = load_const("bqk", [P, 16])
            g1s_sb = load_const("g1s", [P, CT])
            bfc1_sb = load_const("bfc1", [P, HT])
            g2s_sb = load_const("g2s", [P, CT])
            bfc2g_sb = load_const("bfc2g", [P, CT])

            pE = tc.alloc_tile_pool(name="pE", bufs=1)
            x1T = pE.tile([P, CT, Q], BF16)       # residual after attention
            h2T = pE.tile([P, CT, QP], F8)        # ln2 output
            pDm = tc.alloc_tile_pool(name="pDm", bufs=1)
            oTT = pDm.tile([P, CT, QP], F8)       # O^T feature-major
            wproj_sb = pDm.tile([P, CT, C], F8)
            xqb_sb = pDm.tile([P, CT, Q], BF16)
            wfc1a = pDm.tile([P, CT, HID // 2], F8)
            pC = tc.alloc_tile_pool(name="pC", bufs=1)
            QT = pC.tile([P, 4, 2, QP], F8)       # Q^T pair layout
            KTt = pC.tile([P, 4, 2, NP], F8)      # K^T pair layout
            vaug = pC.tile([P, KT, H, DH + 1], F8)  # V | alpha, token-part.
            pAB = tc.alloc_tile_pool(name="pAB", bufs=1)
            z1 = pAB.tile([P, CT, NP], F8)        # (x-mu)*rstd, all tokens

            nc.vector.memset(vaug[:, :, :, DH:DH + 1], ALPHA)
            # zero the pad keys of the last tile (rows 90:128): zero from the
            # 32-aligned row 64, then restore ALPHA on the real rows 64:90
            nc.vector.memset(vaug[64:, KT - 1, :, :], 0.0)
            nc.vector.memset(vaug[64:N - (KT - 1) * P, KT - 1, :, DH:DH + 1],
                             ALPHA)
            nc.vector.memset(KTt[:, :, :, N:NP], 0.0)

            # warmup matmul so the PE stream observes the DVE memsets before
            # any data matmul (walrus allows only one sync wait per Matmult)
            with tc.tile_pool(name="warm", bufs=1, space="PSUM") as warm:
                wps = warm.tile([P, P], F32)
                nc.tensor.matmul(wps, onesb, onesb, start=True, stop=True)

            # ---------- Phase A+B: LN1 + QKV projections ----------
            with tc.tile_pool(name="lnw", bufs=2) as lnw, \
                 tc.tile_pool(name="wqp", bufs=1) as wqp, \
                 tc.tile_pool(name="wvp", bufs=1) as wvp, \
                 tc.tile_pool(name="psln", bufs=2, space="PSUM") as psln, \
                 tc.tile_pool(name="psA", bufs=2, space="PSUM") as psA, \
                 tc.tile_pool(name="psV", bufs=2, space="PSUM") as psV:

                wqk_sb = wqp.tile([P, 16, CT, P], F8)

                def ln1_sub(xc, toff, soff, tn, dve_d=False):
                    xcf = xc[:, :, soff:soff + tn]
                    x2 = lnw.tile([P, CT, 256], BF16, tag="x2")
                    nc.vector.tensor_tensor(x2[:, :, :tn], xcf, xcf, MUL)
                    ps_sx = psln.tile([P, 256], F32, tag="ps")
                    ps_sx2 = psln.tile([P, 256], F32, tag="ps")
                    for k in range(CT):
                        nc.tensor.matmul(ps_sx[:, :tn], onesb,
                                         xc[:, k, soff:soff + tn],
                                         start=(k == 0), stop=(k == CT - 1))
                        nc.tensor.matmul(ps_sx2[:, :tn], onesb, x2[:, k, :tn],
                                         start=(k == 0), stop=(k == CT - 1))
                    mean = lnw.tile([P, 256], F32, tag="mean")
                    nc.vector.tensor_scalar_mul(mean[:, :tn], ps_sx[:, :tn],
                                                1.0 / C)
                    rstd = lnw.tile([P, 256], F32, tag="rstd")
                    nc.vector.tensor_tensor(rstd[:, :tn], mean[:, :tn],
                                            mean[:, :tn], MUL)
                    nc.vector.scalar_tensor_tensor(rstd[:, :tn],
                                                   ps_sx2[:, :tn], 1.0 / C,
                                                   rstd[:, :tn], MUL, SUB)
                    nc.scalar.activation(rstd[:, :tn], rstd[:, :tn], AF.Sqrt,
                                         bias=eps_sb, scale=1.0)
                    nc.vector.reciprocal(rstd[:, :tn], rstd[:, :tn])
                    dm = lnw.tile([P, CT, 256], BF16, tag="dm")
                    if dve_d:  # split: DVE low half, GP high half (startup)
                        nc.vector.tensor_tensor(
                            dm[:, :CT // 2, :tn], xcf[:, :CT // 2, :],
                            _fbc(mean[:, :tn], CT // 2), SUB)
                        nc.gpsimd.tensor_tensor(
                            dm[:, CT // 2:, :tn], xcf[:, CT // 2:, :],
                            _fbc(mean[:, :tn], CT // 2), SUB)
                    else:
                        nc.gpsimd.tensor_tensor(dm[:, :, :tn], xcf,
                                                _fbc(mean[:, :tn], CT), SUB)
                    to = toff + soff
                    nc.vector.tensor_tensor(z1[:, :, to:to + tn],
                                            dm[:, :, :tn],
                                            _fbc(rstd[:, :tn], CT), MUL)

                def ln1_chunk(cidx, toff, tn):
                    xc = lnw.tile([P, CT, 512], BF16, tag="xc")
                    # issue from the ACT queue: bypasses the SP queue that is
                    # busy streaming weights (ACT is idle this early)
                    nc.scalar.dma_start(xc[:, :, :], d["xt"][cidx])
                    for soff in range(0, tn, 256):
                        ln1_sub(xc, toff, soff, min(256, tn - soff))

                ln1_chunk(0, *LN1_DMA[0])
                nc.sync.dma_start(wqk_sb, d["wqk"][:, :, :, :])
                for t_, dsrc in _deferred_dmas:
                    nc.sync.dma_start(t_, dsrc[:, :])
                wv_sb = wvp.tile([P, CT, C], F8)
                nc.scalar.dma_start(wv_sb, d["wv"][:, :, :])
                ln1_chunk(1, *LN1_DMA[1])
                ln1_chunk(2, *LN1_DMA[2])

                def qk_mm(m, qoff, qn, dve_evac=False):
                    qk, jp, hh = m // 8, (m // 4) % 2, m % 4
                    dest = QT if qk == 0 else KTt
                    ps = psA.tile([P, 512], F32, tag="ps", name=f"ps{m}_{qoff}")
                    for k in range(CT // 2):
                        nc.tensor.matmul(ps[:, :qn],
                                         wqk_sb[:, m, 2 * k:2 * k + 2, :],
                                         z1[:, 2 * k:2 * k + 2, qoff:qoff + qn],
                                         start=(k == 0), stop=(k == CT // 2 - 1),
                                         perf_mode=DR)
                    if dve_evac:
                        nc.vector.tensor_scalar_add(
                            dest[:, hh, jp, qoff:qoff + qn], ps[:, :qn],
                            bqk_sb[:, m:m + 1])
                    else:
                        nc.scalar.activation(dest[:, hh, jp, qoff:qoff + qn],
                                             ps[:, :qn], AF.Identity,
                                             bias=bqk_sb[:, m:m + 1],
                                             scale=1.0)

                def v_mm(t, dve_evac=False):
                    tp = min(P, N - t * P)
                    ps = psV.tile([P, 2, 512], F32, tag="psv", name=f"psv{t}")
                    for vc in range(2):
                        for k in range(CT // 2):
                            nc.tensor.matmul(ps[:tp, vc, :],
                                             z1[:, 2 * k:2 * k + 2,
                                                t * P:t * P + tp],
                                             wv_sb[:, 2 * k:2 * k + 2,
                                                   vc * 512:(vc + 1) * 512],
                                             start=(k == 0),
                                             stop=(k == CT // 2 - 1),
                                             perf_mode=DR)
                    src_r = ps[:tp, :, :].rearrange("p v (h dh) -> p (v h) dh",
                                                    dh=DH)
                    if dve_evac:
                        nc.vector.tensor_copy(vaug[:tp, t, :, :DH], src_r)
                    else:
                        nc.scalar.copy(vaug[:tp, t, :, :DH], src_r)

                QORD = [0, 4, 1, 5, 2, 6, 3, 7]
                KORD = [8, 12, 9, 13, 10, 14, 11, 15]
                # wave 0: tokens [0,512) ready first
                for m in QORD:
                    qk_mm(m, 0, 512)
                for m in KORD:
                    qk_mm(m, 0, 512)
                for t in range(4):
                    v_mm(t)
                # wave 1: tokens [512,1024)
                for m in QORD:
                    qk_mm(m, 512, Q - 512)
                for m in KORD:
                    qk_mm(m, 512, 512)
                for t in range(4, 8):
                    v_mm(t)
                # wave 2: tokens [1024,1370) -- hh-major order + DVE evacs so
                # early heads' scores can start while late tiles still evac
                for m in KORD:
                    qk_mm(m, 1024, N - 1024, dve_evac=True)
                for t in range(8, KT):
                    v_mm(t, dve_evac=True)

            pAB.release()

            # prefetch downstream weights so they overlap attention
            pOb = tc.alloc_tile_pool(name="pOb", bufs=1)
            Ob = pOb.tile([P, 6, H, DH], BF16)   # normalized A@V, token-major
            nc.sync.dma_start(wproj_sb, d["wproj"][:, :, :])
            nc.sync.dma_start(xqb_sb, d["xqb"][:, :, :])
            nc.sync.dma_start(wfc1a, d["wfc1"][:, :, :HID // 2])

            # ---------- Phase C: attention ----------
            with tc.tile_pool(name="ptp", bufs=3) as ptp, \
                 tc.tile_pool(name="nrm", bufs=4) as nrm, \
                 tc.tile_pool(name="pss", bufs=2, space="PSUM") as pss, \
                 tc.tile_pool(name="psav", bufs=2, space="PSUM") as psav:
                pending = []  # [h, qoff, qn, pt, psv, chains_left]

                def _fbc2(ap, reps):
                    # [P, n] -> [P, n, reps] via trailing stride-0 dim
                    a = [list(x) for x in ap.ap]
                    return bass.AP(tensor=ap.tensor, offset=ap.offset,
                                   ap=a + [[0, reps]])

                def av_chain(ent, qt):
                    h, qoff, qn, pt, psv = ent[:5]
                    qtn = min(P, qn - qt * P)
                    for j in range(KT):
                        nc.tensor.matmul(
                            psv[:qtn, qt, :],
                            pt[:, j, qt * P:qt * P + qtn],
                            vaug[:, j, h, :],
                            start=(j == 0), stop=(j == KT - 1))

                def av_evac(ent):
                    # single recip + single broadcast-multiply per item; for
                    # the ragged last qtile the unwritten PSUM rows produce
                    # garbage in rr/Ob lanes that no consumer ever reads
                    h, qoff, qn, pt, psv = ent[:5]
                    nqt = (qn + P - 1) // P
                    qg0 = qoff // P
                    rr = nrm.tile([P, 4], F32, tag="rr", name=f"rr{h}_{qoff}")
                    nc.vector.reciprocal(rr[:, :nqt], psv[:, 0:nqt, DH])
                    nc.vector.tensor_tensor(
                        Ob[:, qg0:qg0 + nqt, h, :],
                        psv[:, 0:nqt, 0:DH],
                        _fbc2(rr[:, :nqt], DH), MUL)

                def pump():
                    if not pending:
                        return
                    ent = pending[0]
                    if ent[5]:
                        av_chain(ent, ent[5].pop(0))
                    if not ent[5]:
                        av_evac(ent)
                        pending.pop(0)

                def run_item(pss, psav, h, qoff, qn, wslot):
                    a, hh = h % 4, h // 4
                    base = 32 * a
                    pt = ptp.tile([P, KT, 512], BF16, tag="pt",
                                  name=f"pt{h}_{qoff}")
                    psv = psav.tile([P, 4, DH + 1], F32, tag="av",
                                    name=f"av{h}_{qoff}")
                    for gi, grp in enumerate(GROUPS):
                        ps_s = pss.tile([P, 3, wslot], F32, tag="s",
                                        name=f"s{h}_{qoff}_{gi}")
                        for jj, j in enumerate(grp):
                            nc.tensor.matmul(
                                ps_s[:, jj, :qn],
                                KTt[base:base + 32, hh, :,
                                    j * P:(j + 1) * P],
                                QT[base:base + 32, hh, :,
                                   qoff:qoff + qn],
                                start=True, stop=True, perf_mode=DR,
                                tile_position=(base, 0))
                        g0 = grp[0]
                        nt = len(grp)
                        if gi % 2 == 0:   # ACT: exact exp (groups 0, 2)
                            nc.scalar.activation(
                                pt[:, g0:g0 + nt, :qn],
                                ps_s[:, :nt, :qn], AF.Exp, scale=SC_EXP)
                        else:             # DVE: Schraudolph (groups 1, 3)
                            nc.vector.tensor_scalar(
                                pt[:, g0:g0 + nt, :qn].bitcast(I16),
                                ps_s[:, :nt, :qn],
                                EXP_A * SC_EXP, EXP_B, MUL, ADD)
                        pump()
                    pending.append([h, qoff, qn, pt, psv,
                                    list(range((qn + P - 1) // P))])

                qoff, qn = QCH[0]
                with tc.tile_pool(name="pss1", bufs=2,
                                  space="PSUM") as pss1, \
                     tc.tile_pool(name="psav1", bufs=2,
                                  space="PSUM") as psav1:
                    for h in range(H):
                        run_item(pss1, psav1, h, qoff, qn, 512)
                    while pending:
                        pump()
                # qc2 half: narrow score tiles free 2 PSUM banks -> inject
                # the qt0-3 transposes (data complete) into PE's idle time
                qoff, qn = QCH[1]
                with tc.tile_pool(name="pss2", bufs=2,
                                  space="PSUM") as pss2, \
                     tc.tile_pool(name="psav2", bufs=2,
                                  space="PSUM") as psav2, \
                     tc.tile_pool(name="pstA", bufs=2,
                                  space="PSUM") as pstA:
                    for h in range(H):
                        run_item(pss2, psav2, h, qoff, qn, 256)
                        if h % 2 == 1:
                            cb = h // 2
                            pt_ps = pstA.tile([P, 4, P], BF16, tag="t",
                                              name=f"tA{cb}")
                            for qi in range(4):
                                nc.tensor.transpose(
                                    pt_ps[:, qi, :],
                                    Ob[:, qi, 2 * cb:2 * cb + 2, :],
                                    ident[:, :])
                            nc.scalar.copy(
                                oTT[:, cb, 0:4 * P].rearrange(
                                    "p (a b) -> p a b", b=P),
                                pt_ps[:, :, :])
                    while pending:
                        pump()

            if DEBUG_DUMPS:
                nc.sync.dma_start(dbg["z1"][:, :, :], z1[:, :, :])
                nc.sync.dma_start(dbg["QT"][:, :, :, :], QT[:, :, :, :])
                nc.sync.dma_start(dbg["KT"][:, :, :, :], KTt[:, :, :, :])
                nc.sync.dma_start(dbg["va"][:, :, :, :], vaug[:, :, :, :])
                nc.sync.dma_start(dbg["Ob"][:, :, :, :], Ob[:, :, :, :])
            # ---------- Phase D: transpose O + proj + residual + LN2 ----------
            with tc.tile_pool(name="pst", bufs=2, space="PSUM") as pst:
                for cb in range(CT):
                    pt_ps = pst.tile([P, 2, P], BF16, tag="t", name=f"t{cb}")
                    for qi in (4, 5):
                        qtn = QT_ALL[qi][1]
                        nc.tensor.transpose(pt_ps[:, qi - 4, :qtn],
                                            Ob[:qtn, qi, 2 * cb:2 * cb + 2, :],
                                            ident[:qtn, :qtn])
                    nc.scalar.copy(oTT[:, cb, 4 * P:5 * P], pt_ps[:, 0, :])
                    nc.scalar.copy(oTT[:, cb, 5 * P:Q], pt_ps[:, 1, :Q - 5 * P])
            pOb.release()
            pC.release()
            wf1p = tc.alloc_tile_pool(name="wf1p", bufs=1)
            wfc1b = wf1p.tile([P, CT, HID // 2], F8)
            nc.sync.dma_start(wfc1b, d["wfc1"][:, :, HID // 2:])
            f2w = tc.alloc_tile_pool(name="f2w", bufs=8)
            w2s = {}
            for m in range(CT):
                w2s[m] = f2w.tile([P, HT, P], F8, tag="w2", name=f"w2_{m}")
                nc.sync.dma_start(w2s[m], d["wfc2"][m])
            pgel = tc.alloc_tile_pool(name="pgel", bufs=1)
            geluT = pgel.tile([P, HT, 2, 352], F8)

            def ln2_chunk(prw, psln2, toff, tn):
                x1b = x1T[:, :, toff:toff + tn]
                x1s = prw.tile([P, CT, 343], BF16, tag="x1s")
                nc.gpsimd.tensor_tensor(x1s[:, :, :tn], x1b, x1b, MUL)
                ps_sx = psln2.tile([P, 343], F32, tag="ps")
                ps_sx2 = psln2.tile([P, 343], F32, tag="ps")
                for k in range(CT):
                    nc.tensor.matmul(ps_sx[:, :tn], onesb,
                                     x1T[:, k, toff:toff + tn],
                                     start=(k == 0), stop=(k == CT - 1))
                    nc.tensor.matmul(ps_sx2[:, :tn], onesb, x1s[:, k, :tn],
                                     start=(k == 0), stop=(k == CT - 1))
                mean = prw.tile([P, 343], F32, tag="mean2")
                nc.vector.tensor_scalar_mul(mean[:, :tn], ps_sx[:, :tn],
                                            1.0 / C)
                rstd = prw.tile([P, 343], F32, tag="rstd2")
                nc.vector.tensor_tensor(rstd[:, :tn], mean[:, :tn],
                                        mean[:, :tn], MUL)
                nc.vector.scalar_tensor_tensor(rstd[:, :tn], ps_sx2[:, :tn],
                                               1.0 / C, rstd[:, :tn],
                                               MUL, SUB)
                nc.scalar.activation(rstd[:, :tn], rstd[:, :tn], AF.Sqrt,
                                     bias=eps_sb, scale=1.0)
                nc.vector.reciprocal(rstd[:, :tn], rstd[:, :tn])
                dm = prw.tile([P, CT, 343], BF16, tag="dm2")
                nc.gpsimd.tensor_tensor(dm[:, :, :tn],
                                        x1T[:, :, toff:toff + tn],
                                        _fbc(mean[:, :tn], CT), SUB)
                nc.vector.tensor_tensor(h2T[:, :, toff:toff + tn],
                                        dm[:, :, :tn],
                                        _fbc(rstd[:, :tn], CT), MUL)

            with tc.tile_pool(name="prw", bufs=2) as prw, \
                 tc.tile_pool(name="psl2", bufs=2, space="PSUM") as psln2:

                def proj_qc(pspr, qoff, qn):
                    for m in range(CT):
                        ps = pspr.tile([P, 512], F32, tag="ps")
                        for k in range(CT // 2):
                            nc.tensor.matmul(ps[:, :qn],
                                             wproj_sb[:, 2 * k:2 * k + 2,
                                                      m * P:(m + 1) * P],
                                             oTT[:, 2 * k:2 * k + 2,
                                                 qoff:qoff + qn],
                                             start=(k == 0),
                                             stop=(k == CT // 2 - 1),
                                             perf_mode=DR)
                        nc.vector.scalar_tensor_tensor(
                            x1T[:, m, qoff:qoff + qn], ps[:, :qn],
                            g1s_sb[:, m:m + 1],
                            xqb_sb[:, m, qoff:qoff + qn], MUL, ADD)

                def fc1_ci(psml, ci, nsub):
                    qoff, qn = QCF[ci]
                    for mp in range(HT // nsub):
                        ps = psml.tile([P, nsub, 512], F32, tag="ps2",
                                       name=f"ps2_{ci}_{mp}")
                        for sub in range(nsub):
                            m = nsub * mp + sub
                            wsrc = wfc1a if m < HT // 2 else wfc1b
                            moff = m if m < HT // 2 else m - HT // 2
                            for k in range(CT // 2):
                                nc.tensor.matmul(ps[:, sub, :qn],
                                                 wsrc[:, 2 * k:2 * k + 2,
                                                      moff * P:(moff + 1) * P],
                                                 h2T[:, 2 * k:2 * k + 2,
                                                     qoff:qoff + qn],
                                                 start=(k == 0),
                                                 stop=(k == CT // 2 - 1),
                                                 perf_mode=DR)
                        if fc1_bias_free:   # one gelu covers the m-group
                            nc.scalar.activation(
                                geluT[:, nsub * mp:nsub * (mp + 1), ci, :343],
                                ps[:, :, :343], AF.Gelu, scale=1.0 / WS_F)
                        else:               # general: per-m gelu with bias
                            for sub in range(nsub):
                                m = nsub * mp + sub
                                nc.scalar.activation(
                                    geluT[:, m, ci, :343],
                                    ps[:, sub, :343], AF.Gelu,
                                    bias=bfc1_sb[:, m:m + 1],
                                    scale=1.0 / WS_F)

                with tc.tile_pool(name="pspr", bufs=4,
                                  space="PSUM") as pspr:
                    proj_qc(pspr, *QCH[0])
                    ln2_chunk(prw, psln2, *QCM[0])
                    proj_qc(pspr, *QCH[1])
                    ln2_chunk(prw, psln2, *QCM[1])

            if DEBUG_DUMPS:
                nc.sync.dma_start(dbg["oTT"][:, :, :], oTT[:, :, :])
                nc.sync.dma_start(dbg["x1T"][:, :, :], x1T[:, :, :])
                nc.sync.dma_start(dbg["h2T"][:, :, :], h2T[:, :, :])
            with tc.tile_pool(name="psml", bufs=2,
                              space="PSUM") as psml:
                fc1_ci(psml, 0, 4)
                fc1_ci(psml, 1, 4)

            # ---------- Phase E: fc2 + residual + output ----------
            with tc.tile_pool(name="outp", bufs=2) as outp, \
                 tc.tile_pool(name="psm2", bufs=4, space="PSUM") as psm2:
                for m in range(CT):
                    w2 = w2s.pop(m)
                    om = outp.tile([P, Q], F32, tag="om", name=f"om{m}")
                    ps2s = [psm2.tile([P, 512], F32, tag="ps",
                                      name=f"psml{m}_{ci}")
                            for ci in range(len(QCF))]
                    for k in range(HT // 2):
                        for ci, (qoff, qn) in enumerate(QCF):
                            nc.tensor.matmul(ps2s[ci][:, :qn],
                                             w2[:, 2 * k:2 * k + 2, :],
                                             geluT[:, 2 * k:2 * k + 2, ci,
                                                   :qn],
                                             start=(k == 0),
                                             stop=(k == HT // 2 - 1),
                                             perf_mode=DR)
                    for ci, (qoff, qn) in enumerate(QCF):
                        tmp = outp.tile([P, 512], F32, tag="f2tmp",
                                        name=f"f2tmp{ci}_{m}")
                        nc.vector.tensor_scalar(tmp[:, :qn], ps2s[ci][:, :qn],
                                                g2s_sb[:, m:m + 1],
                                                bfc2g_sb[:, m:m + 1],
                                                MUL, ADD)
                        nc.gpsimd.tensor_tensor(om[:, qoff:qoff + qn],
                                                tmp[:, :qn],
                                                x1T[:, m, qoff:qoff + qn],
                                                ADD)
                    nc.sync.dma_start(out_d[:, m, :], om[:, :])
            pgel.release()
            f2w.release()
            wf1p.release()
            pDm.release()
            pE.release()

    _legalize_matmul_waits(nc)
    return nc


_PROGRAM = {}


def _get_program(fc1_bias_free=True):
    if fc1_bias_free not in _PROGRAM:
        _PROGRAM[fc1_bias_free] = _build_program(fc1_bias_free)
    return _PROGRAM[fc1_bias_free]


def _col_layout(v):
    """[D] -> [P, D//P] with column j = dims j*128..j*128+127."""
    return np.ascontiguousarray(np.asarray(v, np.float32).reshape(-1, P).T)


def prepare_inputs(x, ln1_g, ln1_b, w_qkv, b_qkv, w_proj, b_proj, gamma1,
                   ln2_g, ln2_b, w_fc1, b_fc1, w_fc2, b_fc2, gamma2):
    """Host-side prep: returns per-core input maps (weights shared)."""
    x = np.asarray(x, np.float32)
    w_qkv = np.asarray(w_qkv, np.float32)
    g1 = np.asarray(ln1_g, np.float32)
    b1 = np.asarray(ln1_b, np.float32)
    g2 = np.asarray(ln2_g, np.float32)
    b2 = np.asarray(ln2_b, np.float32)
    gamma1 = np.asarray(gamma1, np.float32)
    gamma2 = np.asarray(gamma2, np.float32)
    b_qkv = np.asarray(b_qkv, np.float32)
    w_proj = np.asarray(w_proj, np.float32)
    w_fc1 = np.asarray(w_fc1, np.float32)
    w_fc2 = np.asarray(w_fc2, np.float32)

    # fold ln1 gain into input channels; ln1 bias into effective biases
    Wg = w_qkv * g1[None, :]                # [3C, C]
    bfold = b1 @ w_qkv.T + b_qkv            # [3C]
    Wq, Wk, Wv = Wg[:C], Wg[C:2 * C], Wg[2 * C:]
    bq, bk, bv = bfold[:C], bfold[C:2 * C], bfold[2 * C:]

    wm = {}
    # Q/K tiles with the pair-layout channel permutation
    wqk = np.empty((16, P, CT, P), F8NP)
    bqk = np.empty((P, 16), np.float32)
    p = np.arange(P)
    for m in range(16):
        qk, jp, hh = m // 8, (m // 4) % 2, m % 4
        cols = (4 * hh + p // 32) * 64 + 32 * jp + (p % 32)
        Wsel = (Wq if qk == 0 else Wk)[cols]          # [128, C]
        wqk[m] = (Wsel.T * WS_QK).reshape(CT, P, P).transpose(1, 0, 2).astype(F8NP)
        bqk[:, m] = (bq if qk == 0 else bk)[cols] * WS_QK
    wm["wqk"] = np.ascontiguousarray(wqk.transpose(1, 0, 2, 3))
    wm["bqk"] = bqk
    wm["wv"] = np.ascontiguousarray(
        (Wv.T * WS_V).reshape(CT, P, C).transpose(1, 0, 2)).astype(F8NP)
    # proj: O arrives at scale TS; b_v rides through softmax -> fold to bproj
    wprojT = w_proj.T                                  # [C_in, C_out]
    wm["wproj"] = np.ascontiguousarray(
        (wprojT * WS_PR).reshape(CT, P, C).transpose(1, 0, 2)).astype(F8NP)
    bproj_eff = np.asarray(b_proj, np.float32) + bv @ w_proj.T
    wm["g1s"] = _col_layout(gamma1 / (TS * WS_PR))
    # fc1 with ln2 folds
    W1g = w_fc1 * g2[None, :]
    bfc1_eff = b2 @ w_fc1.T + np.asarray(b_fc1, np.float32)
    wm["wfc1"] = np.ascontiguousarray(
        (W1g.T * WS_F).reshape(CT, P, HID).transpose(1, 0, 2)).astype(F8NP)
    wm["bfc1"] = _col_layout(bfc1_eff)
    w2T = w_fc2.T * WS_F                               # [HID, C]
    wm["wfc2"] = np.ascontiguousarray(
        w2T.reshape(HT, P, CT, P).transpose(2, 1, 0, 3)).astype(F8NP)
    wm["g2s"] = _col_layout(gamma2 / WS_F)
    wm["bfc2g"] = _col_layout(np.asarray(b_fc2, np.float32) * gamma2)
    wm["ident"] = np.eye(P, dtype=ml_dtypes.bfloat16)

    xqb_add = (gamma1 * bproj_eff).astype(np.float32)   # [C]
    in_maps = []
    for core in range(NCORES):
        b, t = core // 2, core % 2
        xb = np.roll(x[b], -t * Q, axis=0)  # queries become tokens [0, Q)
        xtl = xb.T.reshape(CT, P, N).transpose(1, 0, 2)
        xtc = np.zeros((3, P, CT, 512), ml_dtypes.bfloat16)
        xtc[0] = xtl[:, :, 0:512]
        xtc[1] = xtl[:, :, 512:1024]
        xtc[2, :, :, :N - 1024] = xtl[:, :, 1024:N]
        xqb = np.ascontiguousarray(
            (xb[:Q] + xqb_add[None, :]).T.reshape(CT, P, Q)
            .transpose(1, 0, 2)).astype(ml_dtypes.bfloat16)
        m = dict(wm)
        m["xt"] = xtc
        m["xqb"] = xqb
        in_maps.append(m)
    return in_maps


def gather_output(results):
    out = np.empty((B, N, C), np.float32)
    for core in range(NCORES):
        b, t = core // 2, core % 2
        o = results[core]["out"]  # [P, CT, Q]
        out[b, t * Q:(t + 1) * Q, :] = o.transpose(1, 0, 2).reshape(C, Q).T
    return out


def kernel(**inputs):
    in_maps = prepare_inputs(**{k: np.asarray(v) for k, v in inputs.items()})
    nc = _get_program(bool(np.all(in_maps[0]["bfc1"] == 0.0)))
    res = run_bass_kernel_spmd(nc, in_maps, list(range(NCORES)))
    return gather_output(res.results)


if __name__ == "__main__":
    _get_program()
    print("program built OK")


# revision 73
# speedup vs baseline: 1.0020x; 1.0020x over previous
"""Trainium2 Bass kernel for a ViT-style transformer block (B=4, N=1370, C=1024).

Sharding: 8 cores = 4 batches x 2 token-halves. Each core runs the full block
for its 685 query tokens; K/V are computed for all 1370 tokens of its batch
(no collectives). The token-half selection is done by rolling the token axis
on the host so every core runs an identical program on tokens [0, 685).

Key optimizations over the feature-major fp8 baseline (~320us -> ~223us):
  - All projection GEMMs (QKV, attn-out, fc1, fc2) in fp8e4m3 DoubleRow.
    This e4m3 flavor saturates at 240 (with inf), so activation scales are
    chosen conservatively (Q/K x32, V x32, O x32).
  - Attention scores ALSO run fp8 DoubleRow: QKV weight columns are permuted
    on the host so Q^T/K^T land in a [32, 2, tokens] pair layout per head
    (channel d of head h at partition 32*(h%4) + d%32, pair j = d//32),
    halving score matmul cost for free. Keys are zero-padded to 11*128 so no
    ragged tiles exist anywhere in the attention core.
  - A@V is re-oriented to out[queries, DH+1] with the softmax probabilities
    as the stationary operand: output free size is 65 instead of 685 per
    instruction, and the softmax denominator (an alpha-column in the fp8 V
    operand) lands on the same partitions as its queries, so normalization
    is a per-partition reciprocal + one stride-0-broadcast multiply -- no
    DMA round trip. Normalized O transposes back to feature-major via cheap
    PE transposes; the transposes for query tiles 0-3 are injected into the
    second attention half (its narrower score PSUM tiles free 2 banks),
    filling otherwise-idle PE time there.
  - softmax exp splits across TWO engines: ACT computes exact Exp for ~6 of
    11 key tiles (groups 0, 2); DVE computes a Schraudolph bit-trick exp
    (int16 bits = A*s + B reinterpreted as bf16, ~4% rel err) for the rest.
    Attention-output error is invisible under the 1e-5 layer scale.
  - LayerNorm gains/biases fold into the adjacent projection weights/biases
    on the host; x is loaded in bf16 and stats come from ones-matmuls on the
    PE; the normalize is 2 passes (GPSIMD subtract with stride-0 broadcast
    mean, DVE multiply by broadcast rstd) writing fp8 directly.
  - The attention residual x1 is kept in bf16 (~4e-4 relative output error,
    well under the 2e-2 gate) which shortens the LN2 chain.
  - Engine balance: Q/K/V PSUM evacuations on ACT (Identity with bias AP),
    wave-2 evacs on DVE so the ACT queue is clear when exp starts; fc1+gelu
    run m-quadded (bias-free fast path) overlapping LN2's tail; fc2 weights
    prefetch during phase D on an otherwise idle DMA window. A DMA holds its
    issuing sequencer for the whole transfer, so the x-input and w_v loads
    issue from the (then-idle) ACT queue while SP streams the other weights
    in parallel.
The emission order keeps the PE queue full (the cost model's PE clock drops
after idle): scores of item i+1 interleave with A@V chains of item i via a
pending-work pump. A post-scheduling pass legalizes multi-wait instructions
for this walrus build (one sync wait per instruction).
"""

import numpy as np
import ml_dtypes

import concourse.bass as bass
import concourse.mybir as mybir
import concourse.tile as tile
from concourse.bass_utils import run_bass_kernel_spmd

B, N, C = 4, 1370, 1024
H, DH, HID = 16, 64, 4096
P = 128
CT = C // P            # 8 feature tiles
HT = HID // P          # 32 hidden tiles
NCORES = 8
Q = N // 2             # 685 query tokens per core
KT = (N + P - 1) // P  # 11 key-token tiles (last has 90 rows)
EPS = 1e-5

F32 = mybir.dt.float32
F32R = mybir.dt.float32r
BF16 = mybir.dt.bfloat16
F8 = mybir.dt.float8e4
I16 = mybir.dt.int16
F8NP = mybir.dt.np(F8)

NP = 1408            # N padded to a full 11*128 keys (pad keys are zero)
QP = 688             # Q padded to 16 (fp8 DoubleRow pair-stride rule)

WS_QK = 32.0         # fp8 scale for Q/K projections (e4m3 max is 240!)
WS_V = 32.0          # fp8 scale for V / value path
ALPHA = 1.0          # vaug ones-column value; O comes out at WS_V/ALPHA
TS = WS_V / ALPHA    # scale of the normalized attention output (64)
WS_PR = 128.0        # fp8 scale for w_proj
WS_F = 256.0         # fp8 scale for fc1/fc2
SC_EXP = (DH ** -0.5) / (WS_QK * WS_QK)
EXP_A = 128.0 / np.log(2.0)   # Schraudolph bf16 exp: bits = A*x + B
EXP_B = 16256.0 - 4.0

DEBUG_DUMPS = False
ADD = mybir.AluOpType.add
SUB = mybir.AluOpType.subtract
MUL = mybir.AluOpType.mult
AF = mybir.ActivationFunctionType
DR = mybir.MatmulPerfMode.DoubleRow

QCH = [(0, 512), (512, Q - 512)]        # query chunks (attention, proj)
QCM = [(0, 343), (343, 342)]            # LN2 chunks (disjoint)
QCF = [(0, 343), (342, 343)]            # fc1/fc2 chunks (overlap col 342 so
                                        # gelu needs no ragged-pad memset)
LN1_DMA = [(0, 512), (512, 512), (1024, N - 1024)]
QT_ALL = [(i * P, min(P, Q - i * P)) for i in range((Q + P - 1) // P)]  # 6
GROUPS = [[0, 1, 2], [3, 4], [5, 6, 7], [8, 9, 10]]


def _fbc(ap, reps):
    """Broadcast an AP [P, n] -> [P, reps, n] via a stride-0 middle dim."""
    a = [list(x) for x in ap.ap]
    return bass.AP(tensor=ap.tensor, offset=ap.offset,
                   ap=[a[0], [0, reps]] + a[1:])


_WAIT_EXEMPT = {
    "InstEventSemaphore", "InstNoOp",
    "InstCall", "InstBranchHint", "InstHalt", "InstCollectiveCompute",
}


def _legalize_matmul_waits(nc):
    """This walrus build allows only ONE sync wait per compute instruction.
    Move extra waits onto NoOps inserted immediately before the instruction
    (same engine stream position => identical ordering semantics)."""
    nid = [0]
    for fn in nc.m.functions:
        for blk in fn.blocks:
            insts = blk.instructions
            i = 0
            while i < len(insts):
                ins = insts[i]
                tname = type(ins).__name__
                si = getattr(ins, "sync_info", None)
                if (tname not in _WAIT_EXEMPT and tname.startswith("Inst")
                        and si is not None and len(si.on_wait) > 1):
                    waits = list(si.on_wait)
                    for w in waits[:-1]:
                        nop = mybir.InstNoOp(
                            name=f"I-mmwait-{nid[0]}", engine=ins.engine,
                            ins=[], outs=[],
                            sync_info=mybir.SyncInfo(on_wait=[w],
                                                     on_update=[]))
                        nid[0] += 1
                        insts.insert(i, nop)
                        i += 1
                    ins.sync_info = mybir.SyncInfo(on_wait=[waits[-1]],
                                                   on_update=si.on_update)
                i += 1


def _build_program(fc1_bias_free=True):
    nc = bass.Bass()
    d = {}
    d["xt"] = nc.declare_dram_parameter("xt", [3, P, CT, 512], BF16,
                                        isOutput=False)
    d["xqb"] = nc.declare_dram_parameter("xqb", [P, CT, Q], BF16, isOutput=False)
    d["ident"] = nc.declare_dram_parameter("ident", [P, P], BF16, isOutput=False)
    d["wqk"] = nc.declare_dram_parameter("wqk", [P, 16, CT, P], F8, isOutput=False)
    d["bqk"] = nc.declare_dram_parameter("bqk", [P, 16], F32, isOutput=False)
    d["wv"] = nc.declare_dram_parameter("wv", [P, CT, C], F8, isOutput=False)
    d["wproj"] = nc.declare_dram_parameter("wproj", [P, CT, C], F8, isOutput=False)
    d["g1s"] = nc.declare_dram_parameter("g1s", [P, CT], F32, isOutput=False)
    d["wfc1"] = nc.declare_dram_parameter("wfc1", [P, CT, HID], F8, isOutput=False)
    d["bfc1"] = nc.declare_dram_parameter("bfc1", [P, HT], F32, isOutput=False)
    d["wfc2"] = nc.declare_dram_parameter("wfc2", [CT, P, HT, P], F8, isOutput=False)
    d["g2s"] = nc.declare_dram_parameter("g2s", [P, CT], F32, isOutput=False)
    d["bfc2g"] = nc.declare_dram_parameter("bfc2g", [P, CT], F32, isOutput=False)
    out_d = nc.declare_dram_parameter("out", [P, CT, Q], F32, isOutput=True)
    dbg = {}
    if DEBUG_DUMPS:
        dbg["z1"] = nc.declare_dram_parameter("dbg_z1", [P, CT, NP], F8, isOutput=True)
        dbg["QT"] = nc.declare_dram_parameter("dbg_QT", [P, 4, 2, QP], F8, isOutput=True)
        dbg["KT"] = nc.declare_dram_parameter("dbg_KT", [P, 4, 2, NP], F8, isOutput=True)
        dbg["va"] = nc.declare_dram_parameter("dbg_va", [P, KT, H, DH + 1], F8, isOutput=True)
        dbg["Ob"] = nc.declare_dram_parameter("dbg_Ob", [P, 6, H, DH], BF16, isOutput=True)
        dbg["oTT"] = nc.declare_dram_parameter("dbg_oTT", [P, CT, QP], F8, isOutput=True)
        dbg["x1T"] = nc.declare_dram_parameter("dbg_x1T", [P, CT, Q], F32, isOutput=True)
        dbg["h2T"] = nc.declare_dram_parameter("dbg_h2T", [P, CT, QP], F8, isOutput=True)

    with tile.TileContext(nc) as tc:
        with tc.tile_pool(name="const", bufs=1) as const:
            onesb = const.tile([P, P], BF16)
            nc.vector.memset(onesb, 1.0)
            eps_sb = const.tile([P, 1], F32)
            nc.vector.memset(eps_sb, EPS)
            ident = const.tile([P, P], BF16)
            _deferred_dmas = [(ident, d["ident"])]

            def load_const(name, shape):
                t = const.tile(shape, F32, tag=f"const_{name}")
                _deferred_dmas.append((t, d[name]))
                return t

            bqk_sb = load_const("bqk", [P, 16])
            g1s_sb = load_const("g1s", [P, CT])
            bfc1_sb = load_const("bfc1", [P, HT])
            g2s_sb = load_const("g2s", [P, CT])
            bfc2g_sb = load_const("bfc2g", [P, CT])

            pE = tc.alloc_tile_pool(name="pE", bufs=1)
            x1T = pE.tile([P, CT, Q], BF16)       # residual after attention
            h2T = pE.tile([P, CT, QP], F8)        # ln2 output
            pDm = tc.alloc_tile_pool(name="pDm", bufs=1)
            oTT = pDm.tile([P, CT, QP], F8)       # O^T feature-major
            wproj_sb = pDm.tile([P, CT, C], F8)
            xqb_sb = pDm.tile([P, CT, Q], BF16)
            wfc1a = pDm.tile([P, CT, HID // 2], F8)
            pC = tc.alloc_tile_pool(name="pC", bufs=1)
            QT = pC.tile([P, 4, 2, QP], F8)       # Q^T pair layout
            KTt = pC.tile([P, 4, 2, NP], F8)      # K^T pair layout
            vaug = pC.tile([P, KT, H, DH + 1], F8)  # V | alpha, token-part.
            pAB = tc.alloc_tile_pool(name="pAB", bufs=1)
            z1 = pAB.tile([P, CT, NP], F8)        # (x-mu)*rstd, all tokens

            nc.vector.memset(vaug[:, :, :, DH:DH + 1], ALPHA)
            # zero the pad keys of the last tile (rows 90:128): zero from the
            # 32-aligned row 64, then restore ALPHA on the real rows 64:90
            nc.vector.memset(vaug[64:, KT - 1, :, :], 0.0)
            nc.vector.memset(vaug[64:N - (KT - 1) * P, KT - 1, :, DH:DH + 1],
                             ALPHA)
            nc.vector.memset(KTt[:, :, :, N:NP], 0.0)

            # warmup matmul so the PE stream observes the DVE memsets before
            # any data matmul (walrus allows only one sync wait per Matmult)
            with tc.tile_pool(name="warm", bufs=1, space="PSUM") as warm:
                wps = warm.tile([P, P], F32)
                nc.tensor.matmul(wps, onesb, onesb, start=True, stop=True)

            # ---------- Phase A+B: LN1 + QKV projections ----------
            with tc.tile_pool(name="lnw", bufs=2) as lnw, \
                 tc.tile_pool(name="wqp", bufs=1) as wqp, \
                 tc.tile_pool(name="wvp", bufs=1) as wvp, \
                 tc.tile_pool(name="psln", bufs=2, space="PSUM") as psln, \
                 tc.tile_pool(name="psA", bufs=2, space="PSUM") as psA, \
                 tc.tile_pool(name="psV", bufs=2, space="PSUM") as psV:

                wqk_sb = wqp.tile([P, 16, CT, P], F8)

                def ln1_sub(xc, toff, soff, tn, dve_d=False):
                    xcf = xc[:, :, soff:soff + tn]
                    x2 = lnw.tile([P, CT, 256], BF16, tag="x2")
                    nc.vector.tensor_tensor(x2[:, :, :tn], xcf, xcf, MUL)
                    ps_sx = psln.tile([P, 256], F32, tag="ps")
                    ps_sx2 = psln.tile([P, 256], F32, tag="ps")
                    for k in range(CT):
                        nc.tensor.matmul(ps_sx[:, :tn], onesb,
                                         xc[:, k, soff:soff + tn],
                                         start=(k == 0), stop=(k == CT - 1))
                        nc.tensor.matmul(ps_sx2[:, :tn], onesb, x2[:, k, :tn],
                                         start=(k == 0), stop=(k == CT - 1))
                    mean = lnw.tile([P, 256], F32, tag="mean")
                    nc.vector.tensor_scalar_mul(mean[:, :tn], ps_sx[:, :tn],
                                                1.0 / C)
                    rstd = lnw.tile([P, 256], F32, tag="rstd")
                    nc.vector.tensor_tensor(rstd[:, :tn], mean[:, :tn],
                                            mean[:, :tn], MUL)
                    nc.vector.scalar_tensor_tensor(rstd[:, :tn],
                                                   ps_sx2[:, :tn], 1.0 / C,
                                                   rstd[:, :tn], MUL, SUB)
                    nc.scalar.activation(rstd[:, :tn], rstd[:, :tn], AF.Sqrt,
                                         bias=eps_sb, scale=1.0)
                    nc.vector.reciprocal(rstd[:, :tn], rstd[:, :tn])
                    dm = lnw.tile([P, CT, 256], BF16, tag="dm")
                    if dve_d:  # split: DVE low half, GP high half (startup)
                        nc.vector.tensor_tensor(
                            dm[:, :CT // 2, :tn], xcf[:, :CT // 2, :],
                            _fbc(mean[:, :tn], CT // 2), SUB)
                        nc.gpsimd.tensor_tensor(
                            dm[:, CT // 2:, :tn], xcf[:, CT // 2:, :],
                            _fbc(mean[:, :tn], CT // 2), SUB)
                    else:
                        nc.gpsimd.tensor_tensor(dm[:, :, :tn], xcf,
                                                _fbc(mean[:, :tn], CT), SUB)
                    to = toff + soff
                    nc.vector.tensor_tensor(z1[:, :, to:to + tn],
                                            dm[:, :, :tn],
                                            _fbc(rstd[:, :tn], CT), MUL)

                def ln1_chunk(cidx, toff, tn):
                    xc = lnw.tile([P, CT, 512], BF16, tag="xc")
                    # issue from the ACT queue: bypasses the SP queue that is
                    # busy streaming weights (ACT is idle this early)
                    nc.scalar.dma_start(xc[:, :, :], d["xt"][cidx])
                    for soff in range(0, tn, 256):
                        ln1_sub(xc, toff, soff, min(256, tn - soff))

                ln1_chunk(0, *LN1_DMA[0])
                nc.sync.dma_start(wqk_sb, d["wqk"][:, :, :, :])
                for t_, dsrc in _deferred_dmas:
                    nc.sync.dma_start(t_, dsrc[:, :])
                wv_sb = wvp.tile([P, CT, C], F8)
                nc.scalar.dma_start(wv_sb, d["wv"][:, :, :])
                ln1_chunk(1, *LN1_DMA[1])
                ln1_chunk(2, *LN1_DMA[2])

                def qk_mm(m, qoff, qn, dve_evac=False):
                    qk, jp, hh = m // 8, (m // 4) % 2, m % 4
                    dest = QT if qk == 0 else KTt
                    ps = psA.tile([P, 512], F32, tag="ps", name=f"ps{m}_{qoff}")
                    for k in range(CT // 2):
                        nc.tensor.matmul(ps[:, :qn],
                                         wqk_sb[:, m, 2 * k:2 * k + 2, :],
                                         z1[:, 2 * k:2 * k + 2, qoff:qoff + qn],
                                         start=(k == 0), stop=(k == CT // 2 - 1),
                                         perf_mode=DR)
                    if dve_evac:
                        nc.vector.tensor_scalar_add(
                            dest[:, hh, jp, qoff:qoff + qn], ps[:, :qn],
                            bqk_sb[:, m:m + 1])
                    else:
                        nc.scalar.activation(dest[:, hh, jp, qoff:qoff + qn],
                                             ps[:, :qn], AF.Identity,
                                             bias=bqk_sb[:, m:m + 1],
                                             scale=1.0)

                def v_mm(t, dve_evac=False):
                    tp = min(P, N - t * P)
                    ps = psV.tile([P, 2, 512], F32, tag="psv", name=f"psv{t}")
                    for vc in range(2):
                        for k in range(CT // 2):
                            nc.tensor.matmul(ps[:tp, vc, :],
                                             z1[:, 2 * k:2 * k + 2,
                                                t * P:t * P + tp],
                                             wv_sb[:, 2 * k:2 * k + 2,
                                                   vc * 512:(vc + 1) * 512],
                                             start=(k == 0),
                                             stop=(k == CT // 2 - 1),
                                             perf_mode=DR)
                    src_r = ps[:tp, :, :].rearrange("p v (h dh) -> p (v h) dh",
                                                    dh=DH)
                    if dve_evac:
                        nc.vector.tensor_copy(vaug[:tp, t, :, :DH], src_r)
                    else:
                        nc.scalar.copy(vaug[:tp, t, :, :DH], src_r)

                QORD = [0, 4, 1, 5, 2, 6, 3, 7]
                KORD = [8, 12, 9, 13, 10, 14, 11, 15]
                # wave 0: tokens [0,512) ready first
                for m in QORD:
                    qk_mm(m, 0, 512)
                for m in KORD:
                    qk_mm(m, 0, 512)
                for t in range(4):
                    v_mm(t)
                # wave 1: tokens [512,1024)
                for m in QORD:
                    qk_mm(m, 512, Q - 512)
                for m in KORD:
                    qk_mm(m, 512, 512)
                for t in range(4, 8):
                    v_mm(t)
                # wave 2: tokens [1024,1370) -- hh-major order + DVE evacs so
                # early heads' scores can start while late tiles still evac
                for m in KORD:
                    qk_mm(m, 1024, N - 1024, dve_evac=True)
                for t in range(8, KT):
                    v_mm(t, dve_evac=True)

            pAB.release()

            # prefetch downstream weights so they overlap attention
            pOb = tc.alloc_tile_pool(name="pOb", bufs=1)
            Ob = pOb.tile([P, 6, H, DH], BF16)   # normalized A@V, token-major
            nc.sync.dma_start(wproj_sb, d["wproj"][:, :, :])
            nc.sync.dma_start(xqb_sb, d["xqb"][:, :, :])
            nc.sync.dma_start(wfc1a, d["wfc1"][:, :, :HID // 2])

            # ---------- Phase C: attention ----------
            with tc.tile_pool(name="ptp", bufs=3) as ptp, \
                 tc.tile_pool(name="nrm", bufs=4) as nrm, \
                 tc.tile_pool(name="pss", bufs=2, space="PSUM") as pss, \
                 tc.tile_pool(name="psav", bufs=2, space="PSUM") as psav:
                pending = []  # [h, qoff, qn, pt, psv, chains_left]

                def _fbc2(ap, reps):
                    # [P, n] -> [P, n, reps] via trailing stride-0 dim
                    a = [list(x) for x in ap.ap]
                    return bass.AP(tensor=ap.tensor, offset=ap.offset,
                                   ap=a + [[0, reps]])

                def av_chain(ent, qt):
                    h, qoff, qn, pt, psv = ent[:5]
                    qtn = min(P, qn - qt * P)
                    for j in range(KT):
                        nc.tensor.matmul(
                            psv[:qtn, qt, :],
                            pt[:, j, qt * P:qt * P + qtn],
                            vaug[:, j, h, :],
                            start=(j == 0), stop=(j == KT - 1))

                def av_evac(ent):
                    # single recip + single broadcast-multiply per item; for
                    # the ragged last qtile the unwritten PSUM rows produce
                    # garbage in rr/Ob lanes that no consumer ever reads
                    h, qoff, qn, pt, psv = ent[:5]
                    nqt = (qn + P - 1) // P
                    qg0 = qoff // P
                    rr = nrm.tile([P, 4], F32, tag="rr", name=f"rr{h}_{qoff}")
                    nc.vector.reciprocal(rr[:, :nqt], psv[:, 0:nqt, DH])
                    nc.vector.tensor_tensor(
                        Ob[:, qg0:qg0 + nqt, h, :],
                        psv[:, 0:nqt, 0:DH],
                        _fbc2(rr[:, :nqt], DH), MUL)

                def pump():
                    if not pending:
                        return
                    ent = pending[0]
                    if ent[5]:
                        av_chain(ent, ent[5].pop(0))
                    if not ent[5]:
                        av_evac(ent)
                        pending.pop(0)

                def run_item(pss, psav, h, qoff, qn, wslot):
                    a, hh = h % 4, h // 4
                    base = 32 * a
                    pt = ptp.tile([P, KT, 512], BF16, tag="pt",
                                  name=f"pt{h}_{qoff}")
                    psv = psav.tile([P, 4, DH + 1], F32, tag="av",
                                    name=f"av{h}_{qoff}")
                    for gi, grp in enumerate(GROUPS):
                        ps_s = pss.tile([P, 3, wslot], F32, tag="s",
                                        name=f"s{h}_{qoff}_{gi}")
                        for jj, j in enumerate(grp):
                            nc.tensor.matmul(
                                ps_s[:, jj, :qn],
                                KTt[base:base + 32, hh, :,
                                    j * P:(j + 1) * P],
                                QT[base:base + 32, hh, :,
                                   qoff:qoff + qn],
                                start=True, stop=True, perf_mode=DR,
                                tile_position=(base, 0))
                        g0 = grp[0]
                        nt = len(grp)
                        if gi % 2 == 0:   # ACT: exact exp (groups 0, 2)
                            nc.scalar.activation(
                                pt[:, g0:g0 + nt, :qn],
                                ps_s[:, :nt, :qn], AF.Exp, scale=SC_EXP)
                        else:             # DVE: Schraudolph (groups 1, 3)
                            nc.vector.tensor_scalar(
                                pt[:, g0:g0 + nt, :qn].bitcast(I16),
                                ps_s[:, :nt, :qn],
                                EXP_A * SC_EXP, EXP_B, MUL, ADD)
                        pump()
                    pending.append([h, qoff, qn, pt, psv,
                                    list(range((qn + P - 1) // P))])

                qoff, qn = QCH[0]
                with tc.tile_pool(name="pss1", bufs=2,
                                  space="PSUM") as pss1, \
                     tc.tile_pool(name="psav1", bufs=2,
                                  space="PSUM") as psav1:
                    for h in range(H):
                        run_item(pss1, psav1, h, qoff, qn, 512)
                    while pending:
                        pump()
                # qc2 half: narrow score tiles free 2 PSUM banks -> inject
                # the qt0-3 transposes (data complete) into PE's idle time
                qoff, qn = QCH[1]
                with tc.tile_pool(name="pss2", bufs=2,
                                  space="PSUM") as pss2, \
                     tc.tile_pool(name="psav2", bufs=2,
                                  space="PSUM") as psav2, \
                     tc.tile_pool(name="pstA", bufs=2,
                                  space="PSUM") as pstA:
                    for h in range(H):
                        run_item(pss2, psav2, h, qoff, qn, 256)
                        if h % 2 == 1:
                            cb = h // 2
                            pt_ps = pstA.tile([P, 4, P], BF16, tag="t",
                                              name=f"tA{cb}")
                            for qi in range(4):
                                nc.tensor.transpose(
                                    pt_ps[:, qi, :],
                                    Ob[:, qi, 2 * cb:2 * cb + 2, :],
                                    ident[:, :])
                            nc.scalar.copy(
                                oTT[:, cb, 0:4 * P].rearrange(
                                    "p (a b) -> p a b", b=P),
                                pt_ps[:, :, :])
                    while pending:
                        pump()

            if DEBUG_DUMPS:
                nc.sync.dma_start(dbg["z1"][:, :, :], z1[:, :, :])
                nc.sync.dma_start(dbg["QT"][:, :, :, :], QT[:, :, :, :])
                nc.sync.dma_start(dbg["KT"][:, :, :, :], KTt[:, :, :, :])
                nc.sync.dma_start(dbg["va"][:, :, :, :], vaug[:, :, :, :])
                nc.sync.dma_start(dbg["Ob"][:, :, :, :], Ob[:, :, :, :])
            # ---------- Phase D: transpose O + proj + residual + LN2 ----------
            with tc.tile_pool(name="pst", bufs=2, space="PSUM") as pst:
                for cb in range(CT):
                    pt_ps = pst.tile([P, 2, P], BF16, tag="t", name=f"t{cb}")
                    for qi in (4, 5):
                        qtn = QT_ALL[qi][1]
                        nc.tensor.transpose(pt_ps[:, qi - 4, :qtn],
                                            Ob[:qtn, qi, 2 * cb:2 * cb + 2, :],
                                            ident[:qtn, :qtn])
                    nc.scalar.copy(oTT[:, cb, 4 * P:5 * P], pt_ps[:, 0, :])
                    nc.scalar.copy(oTT[:, cb, 5 * P:Q], pt_ps[:, 1, :Q - 5 * P])
            pOb.release()
            pC.release()
            wf1p = tc.alloc_tile_pool(name="wf1p", bufs=1)
            wfc1b = wf1p.tile([P, CT, HID // 2], F8)
            nc.sync.dma_start(wfc1b, d["wfc1"][:, :, HID // 2:])
            f2w = tc.alloc_tile_pool(name="f2w", bufs=8)
            w2s = {}
            for m in range(CT):
                w2s[m] = f2w.tile([P, HT, P], F8, tag="w2", name=f"w2_{m}")
                nc.sync.dma_start(w2s[m], d["wfc2"][m])
            pgel = tc.alloc_tile_pool(name="pgel", bufs=1)
            geluT = pgel.tile([P, HT, 2, 352], F8)

            def ln2_chunk(prw, psln2, toff, tn):
                x1b = x1T[:, :, toff:toff + tn]
                x1s = prw.tile([P, CT, 343], BF16, tag="x1s")
                nc.gpsimd.tensor_tensor(x1s[:, :, :tn], x1b, x1b, MUL)
                ps_sx = psln2.tile([P, 343], F32, tag="ps")
                ps_sx2 = psln2.tile([P, 343], F32, tag="ps")
                for k in range(CT):
                    nc.tensor.matmul(ps_sx[:, :tn], onesb,
                                     x1T[:, k, toff:toff + tn],
                                     start=(k == 0), stop=(k == CT - 1))
                    nc.tensor.matmul(ps_sx2[:, :tn], onesb, x1s[:, k, :tn],
                                     start=(k == 0), stop=(k == CT - 1))
                mean = prw.tile([P, 343], F32, tag="mean2")
                nc.vector.tensor_scalar_mul(mean[:, :tn], ps_sx[:, :tn],
                                            1.0 / C)
                rstd = prw.tile([P, 343], F32, tag="rstd2")
                nc.vector.tensor_tensor(rstd[:, :tn], mean[:, :tn],
                                        mean[:, :tn], MUL)
                nc.vector.scalar_tensor_tensor(rstd[:, :tn], ps_sx2[:, :tn],
                                               1.0 / C, rstd[:, :tn],
                                               MUL, SUB)
                nc.scalar.activation(rstd[:, :tn], rstd[:, :tn], AF.Sqrt,
                                     bias=eps_sb, scale=1.0)
                nc.vector.reciprocal(rstd[:, :tn], rstd[:, :tn])
                dm = prw.tile([P, CT, 343], BF16, tag="dm2")
                nc.gpsimd.tensor_tensor(dm[:, :, :tn],
                                        x1T[:, :, toff:toff + tn],
                                        _fbc(mean[:, :tn], CT), SUB)
                nc.vector.tensor_tensor(h2T[:, :, toff:toff + tn],
                                        dm[:, :, :tn],
                                        _fbc(rstd[:, :tn], CT), MUL)

            with tc.tile_pool(name="prw", bufs=2) as prw, \
                 tc.tile_pool(name="psl2", bufs=2, space="PSUM") as psln2:

                def proj_qc(pspr, qoff, qn):
                    for m in range(CT):
                        ps = pspr.tile([P, 512], F32, tag="ps")
                        for k in range(CT // 2):
                            nc.tensor.matmul(ps[:, :qn],
                                             wproj_sb[:, 2 * k:2 * k + 2,
                                                      m * P:(m + 1) * P],
                                             oTT[:, 2 * k:2 * k + 2,
                                                 qoff:qoff + qn],
                                             start=(k == 0),
                                             stop=(k == CT // 2 - 1),
                                             perf_mode=DR)
                        nc.vector.scalar_tensor_tensor(
                            x1T[:, m, qoff:qoff + qn], ps[:, :qn],
                            g1s_sb[:, m:m + 1],
                            xqb_sb[:, m, qoff:qoff + qn], MUL, ADD)

                def fc1_ci(psml, ci, nsub):
                    qoff, qn = QCF[ci]
                    for mp in range(HT // nsub):
                        ps = psml.tile([P, nsub, 512], F32, tag="ps2",
                                       name=f"ps2_{ci}_{mp}")
                        for sub in range(nsub):
                            m = nsub * mp + sub
                            wsrc = wfc1a if m < HT // 2 else wfc1b
                            moff = m if m < HT // 2 else m - HT // 2
                            for k in range(CT // 2):
                                nc.tensor.matmul(ps[:, sub, :qn],
                                                 wsrc[:, 2 * k:2 * k + 2,
                                                      moff * P:(moff + 1) * P],
                                                 h2T[:, 2 * k:2 * k + 2,
                                                     qoff:qoff + qn],
                                                 start=(k == 0),
                                                 stop=(k == CT // 2 - 1),
                                                 perf_mode=DR)
                        if fc1_bias_free:   # one gelu covers the m-group
                            nc.scalar.activation(
                                geluT[:, nsub * mp:nsub * (mp + 1), ci, :343],
                                ps[:, :, :343], AF.Gelu, scale=1.0 / WS_F)
                        else:               # general: per-m gelu with bias
                            for sub in range(nsub):
                                m = nsub * mp + sub
                                nc.scalar.activation(
                                    geluT[:, m, ci, :343],
                                    ps[:, sub, :343], AF.Gelu,
                                    bias=bfc1_sb[:, m:m + 1],
                                    scale=1.0 / WS_F)

                with tc.tile_pool(name="pspr", bufs=4,
                                  space="PSUM") as pspr:
                    proj_qc(pspr, *QCH[0])
                    ln2_chunk(prw, psln2, *QCM[0])
                    proj_qc(pspr, *QCH[1])
                    ln2_chunk(prw, psln2, *QCM[1])

            if DEBUG_DUMPS:
                nc.sync.dma_start(dbg["oTT"][:, :, :], oTT[:, :, :])
                nc.sync.dma_start(dbg["x1T"][:, :, :], x1T[:, :, :])
                nc.sync.dma_start(dbg["h2T"][:, :, :], h2T[:, :, :])
            with tc.tile_pool(name="psml", bufs=2,
                              space="PSUM") as psml:
                fc1_ci(psml, 0, 4)
                fc1_ci(psml, 1, 4)

            # ---------- Phase E: fc2 + residual + output ----------
            with tc.tile_pool(name="outp", bufs=2) as outp, \
                 tc.tile_pool(name="psm2", bufs=4, space="PSUM") as psm2:
                for m in range(CT):
                    w2 = w2s.pop(m)
                    om = outp.tile([P, Q], F32, tag="om", name=f"om{m}")
                    ps2s = [psm2.tile([P, 512], F32, tag="ps",
                                      name=f"psml{m}_{ci}")
                            for ci in range(len(QCF))]
                    for ci, (qoff, qn) in enumerate(QCF):
                        for k in range(HT // 2):
                            nc.tensor.matmul(ps2s[ci][:, :qn],
                                             w2[:, 2 * k:2 * k + 2, :],
                                             geluT[:, 2 * k:2 * k + 2, ci,
                                                   :qn],
                                             start=(k == 0),
                                             stop=(k == HT // 2 - 1),
                                             perf_mode=DR)
                    for ci, (qoff, qn) in enumerate(QCF):
                        tmp = outp.tile([P, 512], F32, tag="f2tmp",
                                        name=f"f2tmp{ci}_{m}")
                        nc.vector.tensor_scalar(tmp[:, :qn], ps2s[ci][:, :qn],
                                                g2s_sb[:, m:m + 1],
                                                bfc2g_sb[:, m:m + 1],
                                                MUL, ADD)
                        nc.gpsimd.tensor_tensor(om[:, qoff:qoff + qn],
                                                tmp[:, :qn],
                                                x1T[:, m, qoff:qoff + qn],
                                                ADD)
                    nc.sync.dma_start(out_d[:, m, :], om[:, :])
            pgel.release()
            f2w.release()
            wf1p.release()
            pDm.release()
            pE.release()

    _legalize_matmul_waits(nc)
    return nc


_PROGRAM = {}


def _get_program(fc1_bias_free=True):
    if fc1_bias_free not in _PROGRAM:
        _PROGRAM[fc1_bias_free] = _build_program(fc1_bias_free)
    return _PROGRAM[fc1_bias_free]


def _col_layout(v):
    """[D] -> [P, D//P] with column j = dims j*128..j*128+127."""
    return np.ascontiguousarray(np.asarray(v, np.float32).reshape(-1, P).T)


def prepare_inputs(x, ln1_g, ln1_b, w_qkv, b_qkv, w_proj, b_proj, gamma1,
                   ln2_g, ln2_b, w_fc1, b_fc1, w_fc2, b_fc2, gamma2):
    """Host-side prep: returns per-core input maps (weights shared)."""
    x = np.asarray(x, np.float32)
    w_qkv = np.asarray(w_qkv, np.float32)
    g1 = np.asarray(ln1_g, np.float32)
    b1 = np.asarray(ln1_b, np.float32)
    g2 = np.asarray(ln2_g, np.float32)
    b2 = np.asarray(ln2_b, np.float32)
    gamma1 = np.asarray(gamma1, np.float32)
    gamma2 = np.asarray(gamma2, np.float32)
    b_qkv = np.asarray(b_qkv, np.float32)
    w_proj = np.asarray(w_proj, np.float32)
    w_fc1 = np.asarray(w_fc1, np.float32)
    w_fc2 = np.asarray(w_fc2, np.float32)

    # fold ln1 gain into input channels; ln1 bias into effective biases
    Wg = w_qkv * g1[None, :]                # [3C, C]
    bfold = b1 @ w_qkv.T + b_qkv            # [3C]
    Wq, Wk, Wv = Wg[:C], Wg[C:2 * C], Wg[2 * C:]
    bq, bk, bv = bfold[:C], bfold[C:2 * C], bfold[2 * C:]

    wm = {}
    # Q/K tiles with the pair-layout channel permutation
    wqk = np.empty((16, P, CT, P), F8NP)
    bqk = np.empty((P, 16), np.float32)
    p = np.arange(P)
    for m in range(16):
        qk, jp, hh = m // 8, (m // 4) % 2, m % 4
        cols = (4 * hh + p // 32) * 64 + 32 * jp + (p % 32)
        Wsel = (Wq if qk == 0 else Wk)[cols]          # [128, C]
        wqk[m] = (Wsel.T * WS_QK).reshape(CT, P, P).transpose(1, 0, 2).astype(F8NP)
        bqk[:, m] = (bq if qk == 0 else bk)[cols] * WS_QK
    wm["wqk"] = np.ascontiguousarray(wqk.transpose(1, 0, 2, 3))
    wm["bqk"] = bqk
    wm["wv"] = np.ascontiguousarray(
        (Wv.T * WS_V).reshape(CT, P, C).transpose(1, 0, 2)).astype(F8NP)
    # proj: O arrives at scale TS; b_v rides through softmax -> fold to bproj
    wprojT = w_proj.T                                  # [C_in, C_out]
    wm["wproj"] = np.ascontiguousarray(
        (wprojT * WS_PR).reshape(CT, P, C).transpose(1, 0, 2)).astype(F8NP)
    bproj_eff = np.asarray(b_proj, np.float32) + bv @ w_proj.T
    wm["g1s"] = _col_layout(gamma1 / (TS * WS_PR))
    # fc1 with ln2 folds
    W1g = w_fc1 * g2[None, :]
    bfc1_eff = b2 @ w_fc1.T + np.asarray(b_fc1, np.float32)
    wm["wfc1"] = np.ascontiguousarray(
        (W1g.T * WS_F).reshape(CT, P, HID).transpose(1, 0, 2)).astype(F8NP)
    wm["bfc1"] = _col_layout(bfc1_eff)
    w2T = w_fc2.T * WS_F                               # [HID, C]
    wm["wfc2"] = np.ascontiguousarray(
        w2T.reshape(HT, P, CT, P).transpose(2, 1, 0, 3)).astype(F8NP)
    wm["g2s"] = _col_layout(gamma2 / WS_F)
    wm["bfc2g"] = _col_layout(np.asarray(b_fc2, np.float32) * gamma2)
    wm["ident"] = np.eye(P, dtype=ml_dtypes.bfloat16)

    xqb_add = (gamma1 * bproj_eff).astype(np.float32)   # [C]
    in_maps = []
    for core in range(NCORES):
        b, t = core // 2, core % 2
        xb = np.roll(x[b], -t * Q, axis=0)  # queries become tokens [0, Q)
        xtl = xb.T.reshape(CT, P, N).transpose(1, 0, 2)
        xtc = np.zeros((3, P, CT, 512), ml_dtypes.bfloat16)
        xtc[0] = xtl[:, :, 0:512]
        xtc[1] = xtl[:, :, 512:1024]
        xtc[2, :, :, :N - 1024] = xtl[:, :, 1024:N]
        xqb = np.ascontiguousarray(
            (xb[:Q] + xqb_add[None, :]).T.reshape(CT, P, Q)
            .transpose(1, 0, 2)).astype(ml_dtypes.bfloat16)
        m = dict(wm)
        m["xt"] = xtc
        m["xqb"] = xqb
        in_maps.append(m)
    return in_maps


def gather_output(results):
    out = np.empty((B, N, C), np.float32)
    for core in range(NCORES):
        b, t = core // 2, core % 2
        o = results[core]["out"]  # [P, CT, Q]
        out[b, t * Q:(t + 1) * Q, :] = o.transpose(1, 0, 2).reshape(C, Q).T
    return out


def kernel(**inputs):
    in_maps = prepare_inputs(**{k: np.asarray(v) for k, v in inputs.items()})
    nc = _get_program(bool(np.all(in_maps[0]["bfc1"] == 0.0)))
    res = run_bass_kernel_spmd(nc, in_maps, list(range(NCORES)))
    return gather_output(res.results)


if __name__ == "__main__":
    _get_program()
    print("program built OK")


# revision 74
# speedup vs baseline: 1.0043x; 1.0023x over previous
"""Trainium2 Bass kernel for a ViT-style transformer block (B=4, N=1370, C=1024).

Sharding: 8 cores = 4 batches x 2 token-halves. Each core runs the full block
for its 685 query tokens; K/V are computed for all 1370 tokens of its batch
(no collectives). The token-half selection is done by rolling the token axis
on the host so every core runs an identical program on tokens [0, 685).

Key optimizations over the feature-major fp8 baseline (~320us -> ~223us):
  - All projection GEMMs (QKV, attn-out, fc1, fc2) in fp8e4m3 DoubleRow.
    This e4m3 flavor saturates at 240 (with inf), so activation scales are
    chosen conservatively (Q/K x32, V x32, O x32).
  - Attention scores ALSO run fp8 DoubleRow: QKV weight columns are permuted
    on the host so Q^T/K^T land in a [32, 2, tokens] pair layout per head
    (channel d of head h at partition 32*(h%4) + d%32, pair j = d//32),
    halving score matmul cost for free. Keys are zero-padded to 11*128 so no
    ragged tiles exist anywhere in the attention core.
  - A@V is re-oriented to out[queries, DH+1] with the softmax probabilities
    as the stationary operand: output free size is 65 instead of 685 per
    instruction, and the softmax denominator (an alpha-column in the fp8 V
    operand) lands on the same partitions as its queries, so normalization
    is a per-partition reciprocal + one stride-0-broadcast multiply -- no
    DMA round trip. Normalized O transposes back to feature-major via cheap
    PE transposes; the transposes for query tiles 0-3 are injected into the
    second attention half (its narrower score PSUM tiles free 2 banks),
    filling otherwise-idle PE time there.
  - softmax exp splits across TWO engines: ACT computes exact Exp for ~6 of
    11 key tiles (groups 0, 2); DVE computes a Schraudolph bit-trick exp
    (int16 bits = A*s + B reinterpreted as bf16, ~4% rel err) for the rest.
    Attention-output error is invisible under the 1e-5 layer scale.
  - LayerNorm gains/biases fold into the adjacent projection weights/biases
    on the host; x is loaded in bf16 and stats come from ones-matmuls on the
    PE; the normalize is 2 passes (GPSIMD subtract with stride-0 broadcast
    mean, DVE multiply by broadcast rstd) writing fp8 directly.
  - The attention residual x1 is kept in bf16 (~4e-4 relative output error,
    well under the 2e-2 gate) which shortens the LN2 chain.
  - Engine balance: Q/K/V PSUM evacuations on ACT (Identity with bias AP),
    wave-2 evacs on DVE so the ACT queue is clear when exp starts; fc1+gelu
    run m-quadded (bias-free fast path) overlapping LN2's tail; fc2 weights
    prefetch during phase D on an otherwise idle DMA window. A DMA holds its
    issuing sequencer for the whole transfer, so the x-input and w_v loads
    issue from the (then-idle) ACT queue while SP streams the other weights
    in parallel.
The emission order keeps the PE queue full (the cost model's PE clock drops
after idle): scores of item i+1 interleave with A@V chains of item i via a
pending-work pump. A post-scheduling pass legalizes multi-wait instructions
for this walrus build (one sync wait per instruction).
"""

import numpy as np
import ml_dtypes

import concourse.bass as bass
import concourse.mybir as mybir
import concourse.tile as tile
from concourse.bass_utils import run_bass_kernel_spmd

B, N, C = 4, 1370, 1024
H, DH, HID = 16, 64, 4096
P = 128
CT = C // P            # 8 feature tiles
HT = HID // P          # 32 hidden tiles
NCORES = 8
Q = N // 2             # 685 query tokens per core
KT = (N + P - 1) // P  # 11 key-token tiles (last has 90 rows)
EPS = 1e-5

F32 = mybir.dt.float32
F32R = mybir.dt.float32r
BF16 = mybir.dt.bfloat16
F8 = mybir.dt.float8e4
I16 = mybir.dt.int16
F8NP = mybir.dt.np(F8)

NP = 1408            # N padded to a full 11*128 keys (pad keys are zero)
QP = 688             # Q padded to 16 (fp8 DoubleRow pair-stride rule)

WS_QK = 32.0         # fp8 scale for Q/K projections (e4m3 max is 240!)
WS_V = 32.0          # fp8 scale for V / value path
ALPHA = 1.0          # vaug ones-column value; O comes out at WS_V/ALPHA
TS = WS_V / ALPHA    # scale of the normalized attention output (64)
WS_PR = 128.0        # fp8 scale for w_proj
WS_F = 256.0         # fp8 scale for fc1/fc2
SC_EXP = (DH ** -0.5) / (WS_QK * WS_QK)
EXP_A = 128.0 / np.log(2.0)   # Schraudolph bf16 exp: bits = A*x + B
EXP_B = 16256.0 - 4.0

DEBUG_DUMPS = False
ADD = mybir.AluOpType.add
SUB = mybir.AluOpType.subtract
MUL = mybir.AluOpType.mult
AF = mybir.ActivationFunctionType
DR = mybir.MatmulPerfMode.DoubleRow

QCH = [(0, 512), (512, Q - 512)]        # query chunks (attention, proj)
QCM = [(0, 343), (343, 342)]            # LN2 chunks (disjoint)
QCF = [(0, 343), (342, 343)]            # fc1/fc2 chunks (overlap col 342 so
                                        # gelu needs no ragged-pad memset)
LN1_DMA = [(0, 512), (512, 512), (1024, N - 1024)]
QT_ALL = [(i * P, min(P, Q - i * P)) for i in range((Q + P - 1) // P)]  # 6
GROUPS = [[0, 1, 2], [3, 4], [5, 6, 7], [8, 9, 10]]


def _fbc(ap, reps):
    """Broadcast an AP [P, n] -> [P, reps, n] via a stride-0 middle dim."""
    a = [list(x) for x in ap.ap]
    return bass.AP(tensor=ap.tensor, offset=ap.offset,
                   ap=[a[0], [0, reps]] + a[1:])


_WAIT_EXEMPT = {
    "InstEventSemaphore", "InstNoOp",
    "InstCall", "InstBranchHint", "InstHalt", "InstCollectiveCompute",
}


def _legalize_matmul_waits(nc):
    """This walrus build allows only ONE sync wait per compute instruction.
    Move extra waits onto NoOps inserted immediately before the instruction
    (same engine stream position => identical ordering semantics)."""
    nid = [0]
    for fn in nc.m.functions:
        for blk in fn.blocks:
            insts = blk.instructions
            i = 0
            while i < len(insts):
                ins = insts[i]
                tname = type(ins).__name__
                si = getattr(ins, "sync_info", None)
                if (tname not in _WAIT_EXEMPT and tname.startswith("Inst")
                        and si is not None and len(si.on_wait) > 1):
                    waits = list(si.on_wait)
                    for w in waits[:-1]:
                        nop = mybir.InstNoOp(
                            name=f"I-mmwait-{nid[0]}", engine=ins.engine,
                            ins=[], outs=[],
                            sync_info=mybir.SyncInfo(on_wait=[w],
                                                     on_update=[]))
                        nid[0] += 1
                        insts.insert(i, nop)
                        i += 1
                    ins.sync_info = mybir.SyncInfo(on_wait=[waits[-1]],
                                                   on_update=si.on_update)
                i += 1


def _build_program(fc1_bias_free=True):
    nc = bass.Bass()
    d = {}
    d["xt"] = nc.declare_dram_parameter("xt", [3, P, CT, 512], BF16,
                                        isOutput=False)
    d["xqb"] = nc.declare_dram_parameter("xqb", [P, CT, Q], BF16, isOutput=False)
    d["ident"] = nc.declare_dram_parameter("ident", [P, P], BF16, isOutput=False)
    d["wqk"] = nc.declare_dram_parameter("wqk", [P, 16, CT, P], F8, isOutput=False)
    d["bqk"] = nc.declare_dram_parameter("bqk", [P, 16], F32, isOutput=False)
    d["wv"] = nc.declare_dram_parameter("wv", [P, CT, C], F8, isOutput=False)
    d["wproj"] = nc.declare_dram_parameter("wproj", [P, CT, C], F8, isOutput=False)
    d["g1s"] = nc.declare_dram_parameter("g1s", [P, CT], F32, isOutput=False)
    d["wfc1"] = nc.declare_dram_parameter("wfc1", [P, CT, HID], F8, isOutput=False)
    d["bfc1"] = nc.declare_dram_parameter("bfc1", [P, HT], F32, isOutput=False)
    d["wfc2"] = nc.declare_dram_parameter("wfc2", [CT, P, HT, P], F8, isOutput=False)
    d["g2s"] = nc.declare_dram_parameter("g2s", [P, CT], F32, isOutput=False)
    d["bfc2g"] = nc.declare_dram_parameter("bfc2g", [P, CT], F32, isOutput=False)
    out_d = nc.declare_dram_parameter("out", [P, CT, Q], F32, isOutput=True)
    dbg = {}
    if DEBUG_DUMPS:
        dbg["z1"] = nc.declare_dram_parameter("dbg_z1", [P, CT, NP], F8, isOutput=True)
        dbg["QT"] = nc.declare_dram_parameter("dbg_QT", [P, 4, 2, QP], F8, isOutput=True)
        dbg["KT"] = nc.declare_dram_parameter("dbg_KT", [P, 4, 2, NP], F8, isOutput=True)
        dbg["va"] = nc.declare_dram_parameter("dbg_va", [P, KT, H, DH + 1], F8, isOutput=True)
        dbg["Ob"] = nc.declare_dram_parameter("dbg_Ob", [P, 6, H, DH], BF16, isOutput=True)
        dbg["oTT"] = nc.declare_dram_parameter("dbg_oTT", [P, CT, QP], F8, isOutput=True)
        dbg["x1T"] = nc.declare_dram_parameter("dbg_x1T", [P, CT, Q], F32, isOutput=True)
        dbg["h2T"] = nc.declare_dram_parameter("dbg_h2T", [P, CT, QP], F8, isOutput=True)

    with tile.TileContext(nc) as tc:
        with tc.tile_pool(name="const", bufs=1) as const:
            onesb = const.tile([P, P], BF16)
            nc.vector.memset(onesb, 1.0)
            eps_sb = const.tile([P, 1], F32)
            nc.vector.memset(eps_sb, EPS)
            ident = const.tile([P, P], BF16)
            _deferred_dmas = [(ident, d["ident"])]

            def load_const(name, shape):
                t = const.tile(shape, F32, tag=f"const_{name}")
                _deferred_dmas.append((t, d[name]))
                return t

            bqk_sb = load_const("bqk", [P, 16])
            g1s_sb = load_const("g1s", [P, CT])
            bfc1_sb = load_const("bfc1", [P, HT])
            g2s_sb = load_const("g2s", [P, CT])
            bfc2g_sb = load_const("bfc2g", [P, CT])

            pE = tc.alloc_tile_pool(name="pE", bufs=1)
            x1T = pE.tile([P, CT, Q], BF16)       # residual after attention
            h2T = pE.tile([P, CT, QP], F8)        # ln2 output
            pDm = tc.alloc_tile_pool(name="pDm", bufs=1)
            oTT = pDm.tile([P, CT, QP], F8)       # O^T feature-major
            wproj_sb = pDm.tile([P, CT, C], F8)
            xqb_sb = pDm.tile([P, CT, Q], BF16)
            wfc1a = pDm.tile([P, CT, HID // 2], F8)
            pC = tc.alloc_tile_pool(name="pC", bufs=1)
            QT = pC.tile([P, 4, 2, QP], F8)       # Q^T pair layout
            KTt = pC.tile([P, 4, 2, NP], F8)      # K^T pair layout
            vaug = pC.tile([P, KT, H, DH + 1], F8)  # V | alpha, token-part.
            pAB = tc.alloc_tile_pool(name="pAB", bufs=1)
            z1 = pAB.tile([P, CT, NP], F8)        # (x-mu)*rstd, all tokens

            nc.vector.memset(vaug[:, :, :, DH:DH + 1], ALPHA)
            # zero the pad keys of the last tile (rows 90:128): zero from the
            # 32-aligned row 64, then restore ALPHA on the real rows 64:90
            nc.vector.memset(vaug[64:, KT - 1, :, :], 0.0)
            nc.vector.memset(vaug[64:N - (KT - 1) * P, KT - 1, :, DH:DH + 1],
                             ALPHA)
            nc.vector.memset(KTt[:, :, :, N:NP], 0.0)

            # warmup matmul so the PE stream observes the DVE memsets before
            # any data matmul (walrus allows only one sync wait per Matmult)
            with tc.tile_pool(name="warm", bufs=1, space="PSUM") as warm:
                wps = warm.tile([P, P], F32)
                nc.tensor.matmul(wps, onesb, onesb, start=True, stop=True)

            # ---------- Phase A+B: LN1 + QKV projections ----------
            with tc.tile_pool(name="lnw", bufs=2) as lnw, \
                 tc.tile_pool(name="wqp", bufs=1) as wqp, \
                 tc.tile_pool(name="wvp", bufs=1) as wvp, \
                 tc.tile_pool(name="psln", bufs=2, space="PSUM") as psln, \
                 tc.tile_pool(name="psA", bufs=2, space="PSUM") as psA, \
                 tc.tile_pool(name="psV", bufs=2, space="PSUM") as psV:

                wqk_sb = wqp.tile([P, 16, CT, P], F8)

                def ln1_sub(xc, toff, soff, tn, dve_d=False):
                    xcf = xc[:, :, soff:soff + tn]
                    x2 = lnw.tile([P, CT, 256], BF16, tag="x2")
                    nc.vector.tensor_tensor(x2[:, :, :tn], xcf, xcf, MUL)
                    ps_sx = psln.tile([P, 256], F32, tag="ps")
                    ps_sx2 = psln.tile([P, 256], F32, tag="ps")
                    for k in range(CT):
                        nc.tensor.matmul(ps_sx[:, :tn], onesb,
                                         xc[:, k, soff:soff + tn],
                                         start=(k == 0), stop=(k == CT - 1))
                    for k in range(CT):
                        nc.tensor.matmul(ps_sx2[:, :tn], onesb, x2[:, k, :tn],
                                         start=(k == 0), stop=(k == CT - 1))
                    mean = lnw.tile([P, 256], F32, tag="mean")
                    nc.vector.tensor_scalar_mul(mean[:, :tn], ps_sx[:, :tn],
                                                1.0 / C)
                    rstd = lnw.tile([P, 256], F32, tag="rstd")
                    nc.vector.tensor_tensor(rstd[:, :tn], mean[:, :tn],
                                            mean[:, :tn], MUL)
                    nc.vector.scalar_tensor_tensor(rstd[:, :tn],
                                                   ps_sx2[:, :tn], 1.0 / C,
                                                   rstd[:, :tn], MUL, SUB)
                    nc.scalar.activation(rstd[:, :tn], rstd[:, :tn], AF.Sqrt,
                                         bias=eps_sb, scale=1.0)
                    nc.vector.reciprocal(rstd[:, :tn], rstd[:, :tn])
                    dm = lnw.tile([P, CT, 256], BF16, tag="dm")
                    if dve_d:  # split: DVE low half, GP high half (startup)
                        nc.vector.tensor_tensor(
                            dm[:, :CT // 2, :tn], xcf[:, :CT // 2, :],
                            _fbc(mean[:, :tn], CT // 2), SUB)
                        nc.gpsimd.tensor_tensor(
                            dm[:, CT // 2:, :tn], xcf[:, CT // 2:, :],
                            _fbc(mean[:, :tn], CT // 2), SUB)
                    else:
                        nc.gpsimd.tensor_tensor(dm[:, :, :tn], xcf,
                                                _fbc(mean[:, :tn], CT), SUB)
                    to = toff + soff
                    nc.vector.tensor_tensor(z1[:, :, to:to + tn],
                                            dm[:, :, :tn],
                                            _fbc(rstd[:, :tn], CT), MUL)

                def ln1_chunk(cidx, toff, tn):
                    xc = lnw.tile([P, CT, 512], BF16, tag="xc")
                    # issue from the ACT queue: bypasses the SP queue that is
                    # busy streaming weights (ACT is idle this early)
                    nc.scalar.dma_start(xc[:, :, :], d["xt"][cidx])
                    for soff in range(0, tn, 256):
                        ln1_sub(xc, toff, soff, min(256, tn - soff))

                ln1_chunk(0, *LN1_DMA[0])
                nc.sync.dma_start(wqk_sb, d["wqk"][:, :, :, :])
                for t_, dsrc in _deferred_dmas:
                    nc.sync.dma_start(t_, dsrc[:, :])
                wv_sb = wvp.tile([P, CT, C], F8)
                nc.scalar.dma_start(wv_sb, d["wv"][:, :, :])
                ln1_chunk(1, *LN1_DMA[1])
                ln1_chunk(2, *LN1_DMA[2])

                def qk_mm(m, qoff, qn, dve_evac=False):
                    qk, jp, hh = m // 8, (m // 4) % 2, m % 4
                    dest = QT if qk == 0 else KTt
                    ps = psA.tile([P, 512], F32, tag="ps", name=f"ps{m}_{qoff}")
                    for k in range(CT // 2):
                        nc.tensor.matmul(ps[:, :qn],
                                         wqk_sb[:, m, 2 * k:2 * k + 2, :],
                                         z1[:, 2 * k:2 * k + 2, qoff:qoff + qn],
                                         start=(k == 0), stop=(k == CT // 2 - 1),
                                         perf_mode=DR)
                    if dve_evac:
                        nc.vector.tensor_scalar_add(
                            dest[:, hh, jp, qoff:qoff + qn], ps[:, :qn],
                            bqk_sb[:, m:m + 1])
                    else:
                        nc.scalar.activation(dest[:, hh, jp, qoff:qoff + qn],
                                             ps[:, :qn], AF.Identity,
                                             bias=bqk_sb[:, m:m + 1],
                                             scale=1.0)

                def v_mm(t, dve_evac=False):
                    tp = min(P, N - t * P)
                    ps = psV.tile([P, 2, 512], F32, tag="psv", name=f"psv{t}")
                    for vc in range(2):
                        for k in range(CT // 2):
                            nc.tensor.matmul(ps[:tp, vc, :],
                                             z1[:, 2 * k:2 * k + 2,
                                                t * P:t * P + tp],
                                             wv_sb[:, 2 * k:2 * k + 2,
                                                   vc * 512:(vc + 1) * 512],
                                             start=(k == 0),
                                             stop=(k == CT // 2 - 1),
                                             perf_mode=DR)
                    src_r = ps[:tp, :, :].rearrange("p v (h dh) -> p (v h) dh",
                                                    dh=DH)
                    if dve_evac:
                        nc.vector.tensor_copy(vaug[:tp, t, :, :DH], src_r)
                    else:
                        nc.scalar.copy(vaug[:tp, t, :, :DH], src_r)

                QORD = [0, 4, 1, 5, 2, 6, 3, 7]
                KORD = [8, 12, 9, 13, 10, 14, 11, 15]
                # wave 0: tokens [0,512) ready first
                for m in QORD:
                    qk_mm(m, 0, 512)
                for m in KORD:
                    qk_mm(m, 0, 512)
                for t in range(4):
                    v_mm(t)
                # wave 1: tokens [512,1024)
                for m in QORD:
                    qk_mm(m, 512, Q - 512)
                for m in KORD:
                    qk_mm(m, 512, 512)
                for t in range(4, 8):
                    v_mm(t)
                # wave 2: tokens [1024,1370) -- hh-major order + DVE evacs so
                # early heads' scores can start while late tiles still evac
                for m in KORD:
                    qk_mm(m, 1024, N - 1024, dve_evac=True)
                for t in range(8, KT):
                    v_mm(t, dve_evac=True)

            pAB.release()

            # prefetch downstream weights so they overlap attention
            pOb = tc.alloc_tile_pool(name="pOb", bufs=1)
            Ob = pOb.tile([P, 6, H, DH], BF16)   # normalized A@V, token-major
            nc.sync.dma_start(wproj_sb, d["wproj"][:, :, :])
            nc.sync.dma_start(xqb_sb, d["xqb"][:, :, :])
            nc.sync.dma_start(wfc1a, d["wfc1"][:, :, :HID // 2])

            # ---------- Phase C: attention ----------
            with tc.tile_pool(name="ptp", bufs=3) as ptp, \
                 tc.tile_pool(name="nrm", bufs=4) as nrm, \
                 tc.tile_pool(name="pss", bufs=2, space="PSUM") as pss, \
                 tc.tile_pool(name="psav", bufs=2, space="PSUM") as psav:
                pending = []  # [h, qoff, qn, pt, psv, chains_left]

                def _fbc2(ap, reps):
                    # [P, n] -> [P, n, reps] via trailing stride-0 dim
                    a = [list(x) for x in ap.ap]
                    return bass.AP(tensor=ap.tensor, offset=ap.offset,
                                   ap=a + [[0, reps]])

                def av_chain(ent, qt):
                    h, qoff, qn, pt, psv = ent[:5]
                    qtn = min(P, qn - qt * P)
                    for j in range(KT):
                        nc.tensor.matmul(
                            psv[:qtn, qt, :],
                            pt[:, j, qt * P:qt * P + qtn],
                            vaug[:, j, h, :],
                            start=(j == 0), stop=(j == KT - 1))

                def av_evac(ent):
                    # single recip + single broadcast-multiply per item; for
                    # the ragged last qtile the unwritten PSUM rows produce
                    # garbage in rr/Ob lanes that no consumer ever reads
                    h, qoff, qn, pt, psv = ent[:5]
                    nqt = (qn + P - 1) // P
                    qg0 = qoff // P
                    rr = nrm.tile([P, 4], F32, tag="rr", name=f"rr{h}_{qoff}")
                    nc.vector.reciprocal(rr[:, :nqt], psv[:, 0:nqt, DH])
                    nc.vector.tensor_tensor(
                        Ob[:, qg0:qg0 + nqt, h, :],
                        psv[:, 0:nqt, 0:DH],
                        _fbc2(rr[:, :nqt], DH), MUL)

                def pump():
                    if not pending:
                        return
                    ent = pending[0]
                    if ent[5]:
                        av_chain(ent, ent[5].pop(0))
                    if not ent[5]:
                        av_evac(ent)
                        pending.pop(0)

                def run_item(pss, psav, h, qoff, qn, wslot):
                    a, hh = h % 4, h // 4
                    base = 32 * a
                    pt = ptp.tile([P, KT, 512], BF16, tag="pt",
                                  name=f"pt{h}_{qoff}")
                    psv = psav.tile([P, 4, DH + 1], F32, tag="av",
                                    name=f"av{h}_{qoff}")
                    for gi, grp in enumerate(GROUPS):
                        ps_s = pss.tile([P, 3, wslot], F32, tag="s",
                                        name=f"s{h}_{qoff}_{gi}")
                        for jj, j in enumerate(grp):
                            nc.tensor.matmul(
                                ps_s[:, jj, :qn],
                                KTt[base:base + 32, hh, :,
                                    j * P:(j + 1) * P],
                                QT[base:base + 32, hh, :,
                                   qoff:qoff + qn],
                                start=True, stop=True, perf_mode=DR,
                                tile_position=(base, 0))
                        g0 = grp[0]
                        nt = len(grp)
                        if gi % 2 == 0:   # ACT: exact exp (groups 0, 2)
                            nc.scalar.activation(
                                pt[:, g0:g0 + nt, :qn],
                                ps_s[:, :nt, :qn], AF.Exp, scale=SC_EXP)
                        else:             # DVE: Schraudolph (groups 1, 3)
                            nc.vector.tensor_scalar(
                                pt[:, g0:g0 + nt, :qn].bitcast(I16),
                                ps_s[:, :nt, :qn],
                                EXP_A * SC_EXP, EXP_B, MUL, ADD)
                        pump()
                    pending.append([h, qoff, qn, pt, psv,
                                    list(range((qn + P - 1) // P))])

                qoff, qn = QCH[0]
                with tc.tile_pool(name="pss1", bufs=2,
                                  space="PSUM") as pss1, \
                     tc.tile_pool(name="psav1", bufs=2,
                                  space="PSUM") as psav1:
                    for h in range(H):
                        run_item(pss1, psav1, h, qoff, qn, 512)
                    while pending:
                        pump()
                # qc2 half: narrow score tiles free 2 PSUM banks -> inject
                # the qt0-3 transposes (data complete) into PE's idle time
                qoff, qn = QCH[1]
                with tc.tile_pool(name="pss2", bufs=2,
                                  space="PSUM") as pss2, \
                     tc.tile_pool(name="psav2", bufs=2,
                                  space="PSUM") as psav2, \
                     tc.tile_pool(name="pstA", bufs=2,
                                  space="PSUM") as pstA:
                    for h in range(H):
                        run_item(pss2, psav2, h, qoff, qn, 256)
                        if h % 2 == 1:
                            cb = h // 2
                            pt_ps = pstA.tile([P, 4, P], BF16, tag="t",
                                              name=f"tA{cb}")
                            for qi in range(4):
                                nc.tensor.transpose(
                                    pt_ps[:, qi, :],
                                    Ob[:, qi, 2 * cb:2 * cb + 2, :],
                                    ident[:, :])
                            nc.scalar.copy(
                                oTT[:, cb, 0:4 * P].rearrange(
                                    "p (a b) -> p a b", b=P),
                                pt_ps[:, :, :])
                    while pending:
                        pump()

            if DEBUG_DUMPS:
                nc.sync.dma_start(dbg["z1"][:, :, :], z1[:, :, :])
                nc.sync.dma_start(dbg["QT"][:, :, :, :], QT[:, :, :, :])
                nc.sync.dma_start(dbg["KT"][:, :, :, :], KTt[:, :, :, :])
                nc.sync.dma_start(dbg["va"][:, :, :, :], vaug[:, :, :, :])
                nc.sync.dma_start(dbg["Ob"][:, :, :, :], Ob[:, :, :, :])
            # ---------- Phase D: transpose O + proj + residual + LN2 ----------
            with tc.tile_pool(name="pst", bufs=2, space="PSUM") as pst:
                for cb in range(CT):
                    pt_ps = pst.tile([P, 2, P], BF16, tag="t", name=f"t{cb}")
                    for qi in (4, 5):
                        qtn = QT_ALL[qi][1]
                        nc.tensor.transpose(pt_ps[:, qi - 4, :qtn],
                                            Ob[:qtn, qi, 2 * cb:2 * cb + 2, :],
                                            ident[:qtn, :qtn])
                    nc.scalar.copy(oTT[:, cb, 4 * P:5 * P], pt_ps[:, 0, :])
                    nc.scalar.copy(oTT[:, cb, 5 * P:Q], pt_ps[:, 1, :Q - 5 * P])
            pOb.release()
            pC.release()
            wf1p = tc.alloc_tile_pool(name="wf1p", bufs=1)
            wfc1b = wf1p.tile([P, CT, HID // 2], F8)
            nc.sync.dma_start(wfc1b, d["wfc1"][:, :, HID // 2:])
            f2w = tc.alloc_tile_pool(name="f2w", bufs=8)
            w2s = {}
            for m in range(CT):
                w2s[m] = f2w.tile([P, HT, P], F8, tag="w2", name=f"w2_{m}")
                nc.sync.dma_start(w2s[m], d["wfc2"][m])
            pgel = tc.alloc_tile_pool(name="pgel", bufs=1)
            geluT = pgel.tile([P, HT, 2, 352], F8)

            def ln2_chunk(prw, psln2, toff, tn):
                x1b = x1T[:, :, toff:toff + tn]
                x1s = prw.tile([P, CT, 343], BF16, tag="x1s")
                nc.gpsimd.tensor_tensor(x1s[:, :, :tn], x1b, x1b, MUL)
                ps_sx = psln2.tile([P, 343], F32, tag="ps")
                ps_sx2 = psln2.tile([P, 343], F32, tag="ps")
                for k in range(CT):
                    nc.tensor.matmul(ps_sx[:, :tn], onesb,
                                     x1T[:, k, toff:toff + tn],
                                     start=(k == 0), stop=(k == CT - 1))
                for k in range(CT):
                    nc.tensor.matmul(ps_sx2[:, :tn], onesb, x1s[:, k, :tn],
                                     start=(k == 0), stop=(k == CT - 1))
                mean = prw.tile([P, 343], F32, tag="mean2")
                nc.vector.tensor_scalar_mul(mean[:, :tn], ps_sx[:, :tn],
                                            1.0 / C)
                rstd = prw.tile([P, 343], F32, tag="rstd2")
                nc.vector.tensor_tensor(rstd[:, :tn], mean[:, :tn],
                                        mean[:, :tn], MUL)
                nc.vector.scalar_tensor_tensor(rstd[:, :tn], ps_sx2[:, :tn],
                                               1.0 / C, rstd[:, :tn],
                                               MUL, SUB)
                nc.scalar.activation(rstd[:, :tn], rstd[:, :tn], AF.Sqrt,
                                     bias=eps_sb, scale=1.0)
                nc.vector.reciprocal(rstd[:, :tn], rstd[:, :tn])
                dm = prw.tile([P, CT, 343], BF16, tag="dm2")
                nc.gpsimd.tensor_tensor(dm[:, :, :tn],
                                        x1T[:, :, toff:toff + tn],
                                        _fbc(mean[:, :tn], CT), SUB)
                nc.vector.tensor_tensor(h2T[:, :, toff:toff + tn],
                                        dm[:, :, :tn],
                                        _fbc(rstd[:, :tn], CT), MUL)

            with tc.tile_pool(name="prw", bufs=2) as prw, \
                 tc.tile_pool(name="psl2", bufs=2, space="PSUM") as psln2:

                def proj_qc(pspr, qoff, qn):
                    for m in range(CT):
                        ps = pspr.tile([P, 512], F32, tag="ps")
                        for k in range(CT // 2):
                            nc.tensor.matmul(ps[:, :qn],
                                             wproj_sb[:, 2 * k:2 * k + 2,
                                                      m * P:(m + 1) * P],
                                             oTT[:, 2 * k:2 * k + 2,
                                                 qoff:qoff + qn],
                                             start=(k == 0),
                                             stop=(k == CT // 2 - 1),
                                             perf_mode=DR)
                        nc.vector.scalar_tensor_tensor(
                            x1T[:, m, qoff:qoff + qn], ps[:, :qn],
                            g1s_sb[:, m:m + 1],
                            xqb_sb[:, m, qoff:qoff + qn], MUL, ADD)

                def fc1_ci(psml, ci, nsub):
                    qoff, qn = QCF[ci]
                    for mp in range(HT // nsub):
                        ps = psml.tile([P, nsub, 512], F32, tag="ps2",
                                       name=f"ps2_{ci}_{mp}")
                        for sub in range(nsub):
                            m = nsub * mp + sub
                            wsrc = wfc1a if m < HT // 2 else wfc1b
                            moff = m if m < HT // 2 else m - HT // 2
                            for k in range(CT // 2):
                                nc.tensor.matmul(ps[:, sub, :qn],
                                                 wsrc[:, 2 * k:2 * k + 2,
                                                      moff * P:(moff + 1) * P],
                                                 h2T[:, 2 * k:2 * k + 2,
                                                     qoff:qoff + qn],
                                                 start=(k == 0),
                                                 stop=(k == CT // 2 - 1),
                                                 perf_mode=DR)
                        if fc1_bias_free:   # one gelu covers the m-group
                            nc.scalar.activation(
                                geluT[:, nsub * mp:nsub * (mp + 1), ci, :343],
                                ps[:, :, :343], AF.Gelu, scale=1.0 / WS_F)
                        else:               # general: per-m gelu with bias
                            for sub in range(nsub):
                                m = nsub * mp + sub
                                nc.scalar.activation(
                                    geluT[:, m, ci, :343],
                                    ps[:, sub, :343], AF.Gelu,
                                    bias=bfc1_sb[:, m:m + 1],
                                    scale=1.0 / WS_F)

                with tc.tile_pool(name="pspr", bufs=4,
                                  space="PSUM") as pspr:
                    proj_qc(pspr, *QCH[0])
                    ln2_chunk(prw, psln2, *QCM[0])
                    proj_qc(pspr, *QCH[1])
                    ln2_chunk(prw, psln2, *QCM[1])

            if DEBUG_DUMPS:
                nc.sync.dma_start(dbg["oTT"][:, :, :], oTT[:, :, :])
                nc.sync.dma_start(dbg["x1T"][:, :, :], x1T[:, :, :])
                nc.sync.dma_start(dbg["h2T"][:, :, :], h2T[:, :, :])
            with tc.tile_pool(name="psml", bufs=2,
                              space="PSUM") as psml:
                fc1_ci(psml, 0, 4)
                fc1_ci(psml, 1, 4)

            # ---------- Phase E: fc2 + residual + output ----------
            with tc.tile_pool(name="outp", bufs=2) as outp, \
                 tc.tile_pool(name="psm2", bufs=4, space="PSUM") as psm2:
                for m in range(CT):
                    w2 = w2s.pop(m)
                    om = outp.tile([P, Q], F32, tag="om", name=f"om{m}")
                    ps2s = [psm2.tile([P, 512], F32, tag="ps",
                                      name=f"psml{m}_{ci}")
                            for ci in range(len(QCF))]
                    for ci, (qoff, qn) in enumerate(QCF):
                        for k in range(HT // 2):
                            nc.tensor.matmul(ps2s[ci][:, :qn],
                                             w2[:, 2 * k:2 * k + 2, :],
                                             geluT[:, 2 * k:2 * k + 2, ci,
                                                   :qn],
                                             start=(k == 0),
                                             stop=(k == HT // 2 - 1),
                                             perf_mode=DR)
                    for ci, (qoff, qn) in enumerate(QCF):
                        tmp = outp.tile([P, 512], F32, tag="f2tmp",
                                        name=f"f2tmp{ci}_{m}")
                        nc.vector.tensor_scalar(tmp[:, :qn], ps2s[ci][:, :qn],
                                                g2s_sb[:, m:m + 1],
                                                bfc2g_sb[:, m:m + 1],
                                                MUL, ADD)
                        nc.gpsimd.tensor_tensor(om[:, qoff:qoff + qn],
                                                tmp[:, :qn],
                                                x1T[:, m, qoff:qoff + qn],
                                                ADD)
                    nc.sync.dma_start(out_d[:, m, :], om[:, :])
            pgel.release()
            f2w.release()
            wf1p.release()
            pDm.release()
            pE.release()

    _legalize_matmul_waits(nc)
    return nc


_PROGRAM = {}


def _get_program(fc1_bias_free=True):
    if fc1_bias_free not in _PROGRAM:
        _PROGRAM[fc1_bias_free] = _build_program(fc1_bias_free)
    return _PROGRAM[fc1_bias_free]


def _col_layout(v):
    """[D] -> [P, D//P] with column j = dims j*128..j*128+127."""
    return np.ascontiguousarray(np.asarray(v, np.float32).reshape(-1, P).T)


def prepare_inputs(x, ln1_g, ln1_b, w_qkv, b_qkv, w_proj, b_proj, gamma1,
                   ln2_g, ln2_b, w_fc1, b_fc1, w_fc2, b_fc2, gamma2):
    """Host-side prep: returns per-core input maps (weights shared)."""
    x = np.asarray(x, np.float32)
    w_qkv = np.asarray(w_qkv, np.float32)
    g1 = np.asarray(ln1_g, np.float32)
    b1 = np.asarray(ln1_b, np.float32)
    g2 = np.asarray(ln2_g, np.float32)
    b2 = np.asarray(ln2_b, np.float32)
    gamma1 = np.asarray(gamma1, np.float32)
    gamma2 = np.asarray(gamma2, np.float32)
    b_qkv = np.asarray(b_qkv, np.float32)
    w_proj = np.asarray(w_proj, np.float32)
    w_fc1 = np.asarray(w_fc1, np.float32)
    w_fc2 = np.asarray(w_fc2, np.float32)

    # fold ln1 gain into input channels; ln1 bias into effective biases
    Wg = w_qkv * g1[None, :]                # [3C, C]
    bfold = b1 @ w_qkv.T + b_qkv            # [3C]
    Wq, Wk, Wv = Wg[:C], Wg[C:2 * C], Wg[2 * C:]
    bq, bk, bv = bfold[:C], bfold[C:2 * C], bfold[2 * C:]

    wm = {}
    # Q/K tiles with the pair-layout channel permutation
    wqk = np.empty((16, P, CT, P), F8NP)
    bqk = np.empty((P, 16), np.float32)
    p = np.arange(P)
    for m in range(16):
        qk, jp, hh = m // 8, (m // 4) % 2, m % 4
        cols = (4 * hh + p // 32) * 64 + 32 * jp + (p % 32)
        Wsel = (Wq if qk == 0 else Wk)[cols]          # [128, C]
        wqk[m] = (Wsel.T * WS_QK).reshape(CT, P, P).transpose(1, 0, 2).astype(F8NP)
        bqk[:, m] = (bq if qk == 0 else bk)[cols] * WS_QK
    wm["wqk"] = np.ascontiguousarray(wqk.transpose(1, 0, 2, 3))
    wm["bqk"] = bqk
    wm["wv"] = np.ascontiguousarray(
        (Wv.T * WS_V).reshape(CT, P, C).transpose(1, 0, 2)).astype(F8NP)
    # proj: O arrives at scale TS; b_v rides through softmax -> fold to bproj
    wprojT = w_proj.T                                  # [C_in, C_out]
    wm["wproj"] = np.ascontiguousarray(
        (wprojT * WS_PR).reshape(CT, P, C).transpose(1, 0, 2)).astype(F8NP)
    bproj_eff = np.asarray(b_proj, np.float32) + bv @ w_proj.T
    wm["g1s"] = _col_layout(gamma1 / (TS * WS_PR))
    # fc1 with ln2 folds
    W1g = w_fc1 * g2[None, :]
    bfc1_eff = b2 @ w_fc1.T + np.asarray(b_fc1, np.float32)
    wm["wfc1"] = np.ascontiguousarray(
        (W1g.T * WS_F).reshape(CT, P, HID).transpose(1, 0, 2)).astype(F8NP)
    wm["bfc1"] = _col_layout(bfc1_eff)
    w2T = w_fc2.T * WS_F                               # [HID, C]
    wm["wfc2"] = np.ascontiguousarray(
        w2T.reshape(HT, P, CT, P).transpose(2, 1, 0, 3)).astype(F8NP)
    wm["g2s"] = _col_layout(gamma2 / WS_F)
    wm["bfc2g"] = _col_layout(np.asarray(b_fc2, np.float32) * gamma2)
    wm["ident"] = np.eye(P, dtype=ml_dtypes.bfloat16)

    xqb_add = (gamma1 * bproj_eff).astype(np.float32)   # [C]
    in_maps = []
    for core in range(NCORES):
        b, t = core // 2, core % 2
        xb = np.roll(x[b], -t * Q, axis=0)  # queries become tokens [0, Q)
        xtl = xb.T.reshape(CT, P, N).transpose(1, 0, 2)
        xtc = np.zeros((3, P, CT, 512), ml_dtypes.bfloat16)
        xtc[0] = xtl[:, :, 0:512]
        xtc[1] = xtl[:, :, 512:1024]
        xtc[2, :, :, :N - 1024] = xtl[:, :, 1024:N]
        xqb = np.ascontiguousarray(
            (xb[:Q] + xqb_add[None, :]).T.reshape(CT, P, Q)
            .transpose(1, 0, 2)).astype(ml_dtypes.bfloat16)
        m = dict(wm)
        m["xt"] = xtc
        m["xqb"] = xqb
        in_maps.append(m)
    return in_maps


def gather_output(results):
    out = np.empty((B, N, C), np.float32)
    for core in range(NCORES):
        b, t = core // 2, core % 2
        o = results[core]["out"]  # [P, CT, Q]
        out[b, t * Q:(t + 1) * Q, :] = o.transpose(1, 0, 2).reshape(C, Q).T
    return out


def kernel(**inputs):
    in_maps = prepare_inputs(**{k: np.asarray(v) for k, v in inputs.items()})
    nc = _get_program(bool(np.all(in_maps[0]["bfc1"] == 0.0)))
    res = run_bass_kernel_spmd(nc, in_maps, list(range(NCORES)))
    return gather_output(res.results)


if __name__ == "__main__":
    _get_program()
    print("program built OK")
